# revision 11
# baseline (speedup 1.0000x reference)
"""Trainium2 Bass kernel for a CoaT-style decoder block (ConvPosEnc +
FactorAttn w/ ConvRelPosEnc + FFN), data-parallel over batch on 8 cores.

Layout: activations channel-major [C(part), N(free)]. Matmuls are
weight-stationary (lhsT = W[cin, cout]) so outputs stay channel-major; the
kv einsum uses PE-transposed token-major tiles. Depthwise convs run on the
PE as per-tap diagonal matmuls over spatially shifted access patterns.
Matmul inputs are bf16 (fp32 PSUM accumulation); the residual stream lives
in-place in fp32 tiles; softmax and LN statistics are fp32.

SBUF tag plan (static allocation = sum over tags): the big [128,3136]
tiles share tag groups whose tenants have disjoint lifetimes:
  res{ct}  f32 : x^T -> x0 -> x0+attn -> out  (in-place residual)
  g1{ct} bf16  : xTb -> qT -> wfc1
  g2{ct} bf16  : x0b/x0s(in-place LN) -> attnT -> x0pb/y2(in-place LN)
  g3{ct} bf16  : sq -> ekT -> sq2 -> wfc2
  g4{ct} bf16  : vT  (g4_0 also hosts hdn [128,16,448])
  g5{0,1} bf16 : rbc/mbc -> wqkv01/23 -> ektok/vtok -> rbc2/mbc2
"""

import numpy as np
import ml_dtypes

import concourse.bass as bass
import concourse.bacc as bacc
import concourse.tile as tile
import concourse.mybir as mybir
from concourse import bass_utils

F32 = mybir.dt.float32
BF16 = mybir.dt.bfloat16
AF = mybir.ActivationFunctionType
OP = mybir.AluOpType
AX = mybir.AxisListType

B, NTOK, C = 16, 3136, 512
HH = WW = 56
NHEADS, CHD = 8, 64
HID = 2048
NCORES = 8
BPC = B // NCORES          # images per core
CT = 4                     # 128-channel tiles in C
CHUNK = 448                # tokens per matmul psum chunk (8 image rows)
NCHUNK = NTOK // CHUNK     # 7
RPC = 8                    # image rows per chunk
JT = 25                    # 128-token blocks (last has 64)
EPS = 1e-6

bf16 = ml_dtypes.bfloat16


def _taps(k):
    """Center-first tap list (first matmul must cover the full psum chunk)."""
    p = k // 2
    out = [(0, 0)]
    for dy in range(-p, p + 1):
        for dx in range(-p, p + 1):
            if (dy, dx) != (0, 0):
                out.append((dy, dx))
    return out

TAPS3, TAPS5, TAPS7 = _taps(3), _taps(5), _taps(7)
CRPE_TAPS = [TAPS3, TAPS5, TAPS7, TAPS7]
CRPE_OFF = [0, 9, 34, 83]
CRPE_NTAP = 132


def _diag_pack(ntap, weight_cols):
    out = np.zeros((128, ntap, 128), np.float32)
    idx = np.arange(128)
    for t in range(ntap):
        out[idx, t, idx] = weight_cols[t]
    return out.astype(bf16)


def _prep(inputs):
    g = lambda k: np.asarray(inputs[k], np.float32)
    x = g("x")
    qkv_w, proj_w, proj_b = g("qkv_w"), g("proj_w"), g("proj_b")
    fc1_w, fc1_b, fc2_w, fc2_b = g("fc1_w"), g("fc1_b"), g("fc2_w"), g("fc2_b")
    ln1_w, ln1_b, ln2_w, ln2_b = g("ln1_w"), g("ln1_b"), g("ln2_w"), g("ln2_b")
    cpe_w, cpe_b = g("cpe_w"), g("cpe_b")
    w3, b3, w5, b5, w7, b7 = g("w3"), g("b3"), g("w5"), g("b5"), g("w7"), g("b7")

    wqkv = (ln1_w[:, None] * qkv_w).astype(bf16)
    bqkv = ln1_b @ qkv_w
    wfc1 = (ln2_w[:, None] * fc1_w).astype(bf16)
    bfc1 = fc1_b + ln2_b @ fc1_w

    tiles = lambda b: np.ascontiguousarray(b.reshape(-1, 128).T)

    dcpe = np.concatenate(
        [_diag_pack(9, np.stack([cpe_w[ct * 128:(ct + 1) * 128, 0, dy + 1, dx + 1]
                                 for (dy, dx) in TAPS3]))
         for ct in range(CT)], axis=1)

    def crpe_cols(ct, taps):
        cols = []
        for (dy, dx) in taps:
            w = np.zeros(128, np.float32)
            for p in range(128):
                vch = ct * 128 + p
                if vch < 128:
                    if abs(dy) <= 1 and abs(dx) <= 1:
                        w[p] = w3[vch, 0, dy + 1, dx + 1]
                elif vch < 320:
                    if abs(dy) <= 2 and abs(dx) <= 2:
                        w[p] = w5[vch - 128, 0, dy + 2, dx + 2]
                else:
                    w[p] = w7[vch - 320, 0, dy + 3, dx + 3]
            cols.append(w)
        return np.stack(cols)

    dcrpe = np.concatenate(
        [_diag_pack(len(CRPE_TAPS[ct]), crpe_cols(ct, CRPE_TAPS[ct]))
         for ct in range(CT)], axis=1)

    w = {
        "wqkv": wqkv, "wproj": proj_w.astype(bf16),
        "wfc1": wfc1, "wfc2": fc2_w.astype(bf16),
        "bqkv": tiles(bqkv), "bproj": tiles(proj_b),
        "bfc1": tiles(bfc1), "bfc2": tiles(fc2_b),
        "bcpe": tiles(cpe_b), "bcrpe": tiles(np.concatenate([b3, b5, b7])),
        "dcpe": dcpe, "dcrpe": dcrpe,
        "id_f32": np.eye(128, dtype=np.float32),
        "id_bf": np.eye(128, dtype=np.float32).astype(bf16),
        "ones_col": np.ones((128, 1), bf16),
        "ones_row": np.ones((1, 128), bf16),
    }
    return x, w


WEIGHT_SPECS = [
    ("wqkv", [C, 3 * C], BF16), ("wproj", [C, C], BF16),
    ("wfc1", [C, HID], BF16), ("wfc2", [HID, C], BF16),
    ("bqkv", [128, 12], F32), ("bproj", [128, 4], F32),
    ("bfc1", [128, 16], F32), ("bfc2", [128, 4], F32),
    ("bcpe", [128, 4], F32), ("bcrpe", [128, 4], F32),
    ("dcpe", [128, 36, 128], BF16), ("dcrpe", [128, CRPE_NTAP, 128], BF16),
    ("id_f32", [128, 128], F32), ("id_bf", [128, 128], BF16),
    ("ones_col", [128, 1], BF16), ("ones_row", [1, 128], BF16),
]

# token groups of four 128-blocks (used by transposes); last group is [24]
JGROUPS = [list(range(4 * g, 4 * g + 4)) for g in range(6)] + [[24]]
JW = lambda j: 128 if j < 24 else 64


class Builder:
    def __init__(self, nc, tc, aps, debug):
        self.nc, self.tc, self.aps, self.debug = nc, tc, aps, debug
        self.pools = {}

    def pool(self, name, bufs, space="SBUF"):
        if name not in self.pools:
            self.pools[name] = self.tc.alloc_tile_pool(name=name, bufs=bufs,
                                                       space=space)
        return self.pools[name]

    def dma(self, out, in_):
        self.nc.sync.dma_start(out=out, in_=in_)

    def big(self, tag, dtype=BF16, shape=None):
        return self.pool("pbig", 1).tile(shape or [128, NTOK], dtype,
                                         name=tag, tag=tag)

    def load_weights(self):
        nc, aps = self.nc, self.aps
        pw = self.pool("pw", 1)
        W = {}
        for ci in range(CT):
            t = pw.tile([128, C], BF16, name=f"wproj{ci}", tag=f"wproj{ci}")
            self.dma(t, aps["wproj"][ci * 128:(ci + 1) * 128, :])
            W[f"wproj{ci}"] = t
        for nm in ["bqkv", "bproj", "bfc1", "bfc2", "bcpe", "bcrpe",
                   "id_f32", "id_bf", "ones_col", "ones_row"]:
            t = pw.tile(list(aps[nm].shape), aps[nm].dtype, name=nm, tag=nm)
            self.dma(t, aps[nm])
            W[nm] = t
        eps = pw.tile([128, 1], F32, name="eps", tag="eps")
        nc.vector.memset(eps, EPS)
        W["eps"] = eps
        self.W = W

    def dump(self, name, tiles):
        if not self.debug or name not in self.aps:
            return
        for i, t in enumerate(tiles):
            self.dma(self.aps[name][i], t)

    # ---------- stages ----------
    def transpose_in(self, img):
        """x[img] -> res (f32 channel-major) and xTb (bf16 copy)."""
        nc, W = self.nc, self.W
        ptok = self.pool("ptok", 1)
        pst = self.pool("pst", 2, space="PSUM")
        res = [self.big(f"res{ct}", F32) for ct in range(CT)]
        xTb = [self.big(f"g1_{ct}") for ct in range(CT)]
        for g, js in enumerate(JGROUPS):
            xtok = []
            for j in js:
                t = ptok.tile([128, C], F32, name=f"xtok{j % 4}", tag=f"xtok{j % 4}")
                self.dma(t[:JW(j)], self.aps["x"][img, j * 128:j * 128 + JW(j), :])
                xtok.append((t, JW(j)))
            for ct in range(CT):
                ps = pst.tile([128, 512], F32, name="tpf", tag="tpf")
                for i, (t, rows) in enumerate(xtok):
                    nc.tensor.transpose(ps[:, i * 128:i * 128 + rows],
                                        t[:rows, ct * 128:(ct + 1) * 128],
                                        W["id_f32"][:rows, :rows])
                width = sum(r for _, r in xtok)
                nc.scalar.copy(out=res[ct][:, g * 512:g * 512 + width],
                               in_=ps[:, :width])
        for ct in range(CT):
            nc.vector.tensor_copy(out=xTb[ct], in_=res[ct])
        return res, xTb

    def conv(self, chunk, diag, tap_list, tap_off, src_view, ps):
        nc = self.nc
        psv = ps.rearrange("p (h w) -> p h w", h=RPC)
        r0 = chunk * RPC
        n = len(tap_list)
        for t, (dy, dx) in enumerate(tap_list):
            y0 = max(r0, -dy)
            y1 = min(r0 + RPC, HH - max(0, dy))
            x0 = max(0, -dx)
            x1 = WW - max(0, dx)
            nc.tensor.matmul(
                psv[:, y0 - r0:y1 - r0, x0:x1],
                diag[:, tap_off + t, :],
                src_view[:, y0 + dy:y1 + dy, x0 + dx:x1 + dx],
                start=(t == 0), stop=(t == n - 1), skip_group_check=True)

    def cpe(self, img, res, xTb):
        """res = res + dwconv3(xTb) + bias (in-place); x0b = bf16(res)."""
        nc, W = self.nc, self.W
        pdiag = self.pool("pdiag", 1)
        pmm = self.pool("pmm", 4, space="PSUM")
        dcpe = pdiag.tile([128, 36, 128], BF16, name="diag", tag="diag")
        self.dma(dcpe, self.aps["dcpe"])
        x0b = [self.big(f"g2_{ct}") for ct in range(CT)]
        for ct in range(CT):
            src = xTb[ct].rearrange("p (h w) -> p h w", h=HH)
            for chunk in range(NCHUNK):
                ps = pmm.tile([128, CHUNK], F32, name="mm", tag="mm")
                self.conv(chunk, dcpe, TAPS3, ct * 9, src, ps)
                sl = bass.ts(chunk, CHUNK)
                nc.vector.scalar_tensor_tensor(
                    out=res[ct][:, sl], in0=ps, scalar=W["bcpe"][:, ct:ct + 1],
                    in1=res[ct][:, sl], op0=OP.add, op1=OP.add)
            nc.vector.tensor_copy(out=x0b[ct], in_=res[ct])
        self.dump(f"x0T_{img}", res)
        return x0b

    def ln(self, img, xb, sq_tags, bc_tags):
        """Channel-major LN over xb (list of 4 bf16 tiles), applied IN-PLACE.
        xb becomes the normalized tensor (gamma/beta folded downstream)."""
        nc, W = self.nc, self.W
        psm = self.pool("psmall", 1)
        pstat = self.pool("pst", 2, space="PSUM")
        sq = [self.big(t) for t in sq_tags]
        for ct in range(CT):
            nc.scalar.square(out=sq[ct], in_=xb[ct])
        # per-token sums over channels -> [128, 25] token-tiled stats
        st = psm.tile([128, JT], F32, name="st", tag="st")
        s2t = psm.tile([128, JT], F32, name="s2t", tag="s2t")
        for dst, srcs in ((st, xb), (s2t, sq)):
            pst_cols = pstat.tile([128, 32], F32, name="stt", tag="tpf")
            for g, js in enumerate(JGROUPS):
                w = sum(JW(j) for j in js)
                ps = pstat.tile([1, 512], F32, name="srow", tag="tpf")
                for ct in range(CT):
                    nc.tensor.matmul(ps[:, :w], W["ones_col"],
                                     srcs[ct][:, g * 512:g * 512 + w],
                                     start=(ct == 0), stop=(ct == CT - 1))
                rowb = psm.tile([1, 512], F32, name="rowb", tag="rowb")
                nc.scalar.copy(out=rowb[:, :w], in_=ps[:, :w])
                for i, j in enumerate(js):
                    nc.tensor.transpose(pst_cols[:JW(j), j:j + 1],
                                        rowb[0:1, i * 128:i * 128 + JW(j)],
                                        W["id_f32"][0:1, 0:1])
            nc.vector.tensor_copy(out=dst, in_=pst_cols[:, :JT])
        ms = psm.tile([128, JT], F32, name="ms", tag="ms")
        var = psm.tile([128, JT], F32, name="var", tag="var")
        rstd = psm.tile([128, JT], F32, name="rstd", tag="rstd")
        mrs = psm.tile([128, JT], F32, name="mrs", tag="mrs")
        nc.vector.tensor_scalar_mul(out=ms, in0=st, scalar1=1.0 / C)
        nc.vector.tensor_scalar_mul(out=var, in0=s2t, scalar1=1.0 / C)
        nc.vector.tensor_mul(out=st, in0=ms, in1=ms)
        nc.vector.tensor_sub(out=var, in0=var, in1=st)
        nc.scalar.activation(out=var, in_=var, func=AF.Sqrt, bias=W["eps"],
                             scale=1.0)
        nc.vector.reciprocal(out=rstd, in_=var)
        nc.vector.tensor_mul(out=mrs, in0=ms, in1=rstd)
        # broadcast rstd/mrs along partitions: [128,25] -> row chunks -> K=1 mm
        rbc = self.big(bc_tags[0])
        mbc = self.big(bc_tags[1])
        for dst, src in ((rbc, rstd), (mbc, mrs)):
            for g, js in enumerate(JGROUPS):
                w = sum(JW(j) for j in js)
                psr = pstat.tile([1, 512], F32, name="srow", tag="tpf")
                off = 0
                for j in js:
                    nc.tensor.transpose(psr[0:1, off:off + JW(j)],
                                        src[:JW(j), j:j + 1],
                                        W["id_f32"][:JW(j), :JW(j)])
                    off += JW(j)
                rowb = psm.tile([1, 512], BF16, name="rowbb", tag="rowbb")
                nc.scalar.copy(out=rowb[:, :w], in_=psr[:, :w])
                psb = pstat.tile([128, 512], F32, name="bc", tag="tpf")
                nc.tensor.matmul(psb[:, :w], W["ones_row"], rowb[0:1, :w],
                                 start=True, stop=True)
                nc.scalar.copy(out=dst[:, g * 512:g * 512 + w], in_=psb[:, :w])
        # apply in place: xb = xb * rbc - mbc
        for ct in range(CT):
            nc.vector.tensor_mul(out=xb[ct], in0=xb[ct], in1=rbc)
            nc.vector.tensor_sub(out=xb[ct], in0=xb[ct], in1=mbc)
        return xb

    def qkv(self, img, x0s):
        nc, W = self.nc, self.W
        psm = self.pool("psmall", 1)
        pmm = self.pool("pmm", 4, space="PSUM")
        wq = [self.big("g5_0", BF16, [128, 2, 3 * C]),
              self.big("g5_1", BF16, [128, 2, 3 * C])]
        for ci in range(CT):
            self.dma(wq[ci // 2][:, ci % 2, :],
                     self.aps["wqkv"][ci * 128:(ci + 1) * 128, :])
        qT = [self.big(f"g1_{t}") for t in range(CT)]
        ekT = [self.big(f"g3_{t}") for t in range(CT)]
        vT = [self.big(f"g4_{t}") if t > 0 else
              self.big("g4_0", BF16, [128, 16, CHUNK]) for t in range(CT)]
        vT[0] = vT[0].rearrange("p a b -> p (a b)")[:, :NTOK]
        sep = [psm.tile([128, NCHUNK], F32, name=f"sep{t}", tag=f"sep{t}")
               for t in range(CT)]
        recip = [psm.tile([128, 1], F32, name=f"rec{t}", tag=f"rec{t}")
                 for t in range(CT)]
        for co in range(12):
            for chunk in range(NCHUNK):
                ps = pmm.tile([128, CHUNK], F32, name="mm", tag="mm")
                for ci in range(CT):
                    nc.tensor.matmul(ps, wq[ci // 2][:, ci % 2,
                                                     co * 128:(co + 1) * 128],
                                     x0s[ci][:, bass.ts(chunk, CHUNK)],
                                     start=(ci == 0), stop=(ci == CT - 1))
                bias = W["bqkv"][:, co:co + 1]
                sl = bass.ts(chunk, CHUNK)
                if co < 4:
                    nc.scalar.activation(out=qT[co][:, sl], in_=ps,
                                         func=AF.Identity, bias=bias, scale=1.0)
                elif co < 8:
                    t = co - 4
                    nc.scalar.activation(out=ekT[t][:, sl], in_=ps, func=AF.Exp,
                                         bias=bias, scale=1.0,
                                         accum_out=sep[t][:, chunk:chunk + 1])
                else:
                    nc.scalar.activation(out=vT[co - 8][:, sl], in_=ps,
                                         func=AF.Identity, bias=bias, scale=1.0)
        for t in range(CT):
            s = psm.tile([128, 1], F32, name=f"sume{t}", tag=f"sume{t}")
            nc.vector.tensor_reduce(out=s, in_=sep[t], axis=AX.X, op=OP.add)
            nc.vector.reciprocal(out=recip[t], in_=s)
        self.dump(f"qT_{img}", qT)
        self.dump(f"ekT_{img}", ekT)
        self.dump(f"vT_{img}", vT)
        return qT, ekT, vT, recip

    def kv(self, img, ekT, vT, recip):
        nc, W = self.nc, self.W
        psm = self.pool("psmall", 1)
        pst = self.pool("pst", 2, space="PSUM")
        kv = []
        scale = CHD ** -0.5
        for t in range(CT):
            ektok = self.big("g5_0", BF16, [128, JT, 128])
            vtok = self.big("g5_1", BF16, [128, JT, 128])
            for src, dst in ((ekT[t], ektok), (vT[t], vtok)):
                for g, js in enumerate(JGROUPS):
                    ps = pst.tile([128, 512], BF16, name="tpb", tag="tpb")
                    for i, j in enumerate(js):
                        nc.tensor.transpose(ps[:JW(j), i * 128:(i + 1) * 128],
                                            src[:, j * 128:j * 128 + JW(j)],
                                            W["id_bf"])
                    for i, j in enumerate(js):
                        nc.vector.tensor_copy(
                            out=dst[:JW(j), j, :],
                            in_=ps[:JW(j), i * 128:(i + 1) * 128])
            ps = pst.tile([128, CHD], F32, name="kvps", tag="tpf")
            for h in range(2):
                for j in range(JT):
                    nc.tensor.matmul(
                        ps[h * 64:h * 64 + 64, :],
                        ektok[:JW(j), j, h * 64:h * 64 + 64],
                        vtok[:JW(j), j, h * 64:h * 64 + 64],
                        start=(j == 0), stop=(j == JT - 1),
                        tile_position=(0, h * 64))
            kvt = psm.tile([128, CHD], BF16, name=f"kv{t}", tag=f"kv{t}")
            nc.vector.tensor_scalar(out=kvt, in0=ps, scalar1=recip[t],
                                    scalar2=scale, op0=OP.mult, op1=OP.mult)
            kv.append(kvt)
        self.dump(f"kv_{img}", kv)
        return kv

    def attn(self, img, qT, vT, kv):
        nc, W = self.nc, self.W
        pdiag = self.pool("pdiag", 1)
        psm = self.pool("psmall", 1)
        pmm = self.pool("pmm", 4, space="PSUM")
        attnT = [self.big(f"g2_{t}") for t in range(CT)]
        for ct in range(CT):
            ntap = len(CRPE_TAPS[ct])
            diag = pdiag.tile([128, 49, 128], BF16, name="diag", tag="diag")
            self.dma(diag[:, :ntap, :],
                     self.aps["dcrpe"][:, CRPE_OFF[ct]:CRPE_OFF[ct] + ntap, :])
            src = vT[ct].rearrange("p (h w) -> p h w", h=HH)
            for chunk in range(NCHUNK):
                sl = bass.ts(chunk, CHUNK)
                ps = pmm.tile([128, CHUNK], F32, name="mm", tag="mm")
                self.conv(chunk, diag, CRPE_TAPS[ct], 0, src, ps)
                tmp = psm.tile([128, CHUNK], BF16, name="tmp", tag="tmp")
                nc.vector.scalar_tensor_tensor(
                    out=tmp, in0=ps, scalar=W["bcrpe"][:, ct:ct + 1],
                    in1=qT[ct][:, sl], op0=OP.add, op1=OP.mult)
                ps2 = pmm.tile([128, CHUNK], F32, name="mm", tag="mm")
                for h in range(2):
                    nc.tensor.matmul(ps2[h * 64:h * 64 + 64, :],
                                     kv[ct][h * 64:h * 64 + 64, :],
                                     qT[ct][h * 64:h * 64 + 64, sl],
                                     start=True, stop=True,
                                     tile_position=(h * 64, h * 64))
                nc.vector.tensor_add(out=attnT[ct][:, sl], in0=ps2, in1=tmp)
        self.dump(f"attnT_{img}", attnT)
        return attnT

    def proj(self, img, attnT, res):
        nc, W = self.nc, self.W
        pmm = self.pool("pmm", 4, space="PSUM")
        for co in range(CT):
            for chunk in range(NCHUNK):
                ps = pmm.tile([128, CHUNK], F32, name="mm", tag="mm")
                for ci in range(CT):
                    nc.tensor.matmul(ps,
                                     W[f"wproj{ci}"][:, co * 128:(co + 1) * 128],
                                     attnT[ci][:, bass.ts(chunk, CHUNK)],
                                     start=(ci == 0), stop=(ci == CT - 1))
                sl = bass.ts(chunk, CHUNK)
                nc.vector.scalar_tensor_tensor(
                    out=res[co][:, sl], in0=ps, scalar=W["bproj"][:, co:co + 1],
                    in1=res[co][:, sl], op0=OP.add, op1=OP.add)
        x0pb = [self.big(f"g2_{t}") for t in range(CT)]
        for co in range(CT):
            nc.vector.tensor_copy(out=x0pb[co], in_=res[co])
        self.dump(f"x0pT_{img}", res)
        return x0pb

    def ffn(self, img, y2, res):
        nc, W = self.nc, self.W
        pmm = self.pool("pmm", 4, space="PSUM")
        wfc1 = [self.big(f"g1_{ci}", BF16, [128, HID]) for ci in range(CT)]
        wfc2 = [self.big(f"g3_{kt}", BF16, [128, 4, C]) for kt in range(CT)]
        for ci in range(CT):
            self.dma(wfc1[ci], self.aps["wfc1"][ci * 128:(ci + 1) * 128, :])
        for kt in range(16):
            self.dma(wfc2[kt // 4][:, kt % 4, :],
                     self.aps["wfc2"][kt * 128:(kt + 1) * 128, :])
        for chunk in range(NCHUNK):
            sl = bass.ts(chunk, CHUNK)
            hdn_ab = [self.big("g4_0", BF16, [128, 8, CHUNK]),
                      self.big("g4_1", BF16, [128, 8, CHUNK])]
            hdn = lambda kt: hdn_ab[kt // 8][:, kt % 8, :]
            for ho in range(16):
                ps = pmm.tile([128, CHUNK], F32, name="mm", tag="mm")
                for ci in range(CT):
                    nc.tensor.matmul(ps, wfc1[ci][:, ho * 128:(ho + 1) * 128],
                                     y2[ci][:, sl],
                                     start=(ci == 0), stop=(ci == CT - 1))
                nc.scalar.activation(out=hdn(ho), in_=ps, func=AF.Gelu,
                                     bias=W["bfc1"][:, ho:ho + 1], scale=1.0)
            for co in range(CT):
                ps = pmm.tile([128, CHUNK], F32, name="mm", tag="mm")
                for kt in range(16):
                    nc.tensor.matmul(ps,
                                     wfc2[kt // 4][:, kt % 4,
                                                   co * 128:(co + 1) * 128],
                                     hdn(kt),
                                     start=(kt == 0), stop=(kt == 15))
                nc.vector.scalar_tensor_tensor(
                    out=res[co][:, sl], in0=ps, scalar=W["bfc2"][:, co:co + 1],
                    in1=res[co][:, sl], op0=OP.add, op1=OP.add)
        self.dump(f"outT_{img}", res)

    def transpose_out(self, img, res):
        nc, W = self.nc, self.W
        ptok = self.pool("ptok", 1)
        pst = self.pool("pst", 2, space="PSUM")
        for j in range(JT):
            rows = JW(j)
            ps = pst.tile([128, 512], F32, name="tpf", tag="tpf")
            for ct in range(CT):
                nc.tensor.transpose(ps[:rows, ct * 128:(ct + 1) * 128],
                                    res[ct][:, j * 128:j * 128 + rows],
                                    W["id_f32"])
            t = ptok.tile([128, C], F32, name=f"xtok{j % 4}", tag=f"xtok{j % 4}")
            nc.scalar.copy(out=t[:rows], in_=ps[:rows])
            self.dma(self.aps["out"][img, j * 128:j * 128 + rows, :], t[:rows])

    def image(self, img):
        res, xTb = self.transpose_in(img)
        x0b = self.cpe(img, res, xTb)
        x0s = self.ln(img, x0b, [f"g3_{t}" for t in range(CT)],
                      ["g5_0", "g5_1"])
        self.dump(f"x0s_{img}", x0s)
        qT, ekT, vT, recip = self.qkv(img, x0s)
        kv = self.kv(img, ekT, vT, recip)
        attnT = self.attn(img, qT, vT, kv)
        x0pb = self.proj(img, attnT, res)
        y2 = self.ln(img, x0pb, [f"g3_{t}" for t in range(CT)],
                     ["g5_0", "g5_1"])
        self.dump(f"y2_{img}", y2)
        self.ffn(img, y2, res)
        self.transpose_out(img, res)

    def build(self):
        self.load_weights()
        for img in range(BPC):
            self.image(img)
        for p in reversed(list(self.pools.values())):
            p.release()


DEBUG_TENSORS = []
for img in range(BPC):
    DEBUG_TENSORS += [
        (f"x0T_{img}", F32), (f"x0s_{img}", BF16), (f"qT_{img}", BF16),
        (f"ekT_{img}", BF16), (f"vT_{img}", BF16), (f"attnT_{img}", BF16),
        (f"x0pT_{img}", F32), (f"y2_{img}", BF16), (f"outT_{img}", F32),
    ]


def build_nc(debug=False):
    nc = bacc.Bacc("TRN2", target_bir_lowering=False, debug=False,
                   num_devices=NCORES)
    aps = {}
    aps["x"] = nc.dram_tensor("x", [BPC, NTOK, C], F32, kind="ExternalInput").ap()
    for name, shape, dt in WEIGHT_SPECS:
        aps[name] = nc.dram_tensor(name, shape, dt, kind="ExternalInput").ap()
    aps["out"] = nc.dram_tensor("out", [BPC, NTOK, C], F32,
                                kind="ExternalOutput").ap()
    if debug:
        for name, dt in DEBUG_TENSORS:
            aps[name] = nc.dram_tensor(name, [CT, 128, NTOK], dt,
                                       kind="ExternalOutput").ap()
        aps["kv_0"] = nc.dram_tensor("kv_0", [CT, 128, CHD], BF16,
                                     kind="ExternalOutput").ap()
    with tile.TileContext(nc) as tc:
        Builder(nc, tc, aps, debug).build()
    nc.compile()
    return nc


_CACHE = {}


def run(inputs, debug=False):
    x, w = _prep(inputs)
    key = "dbg" if debug else "plain"
    if key not in _CACHE:
        _CACHE[key] = build_nc(debug)
    nc = _CACHE[key]
    in_maps = []
    for c in range(NCORES):
        m = {"x": np.ascontiguousarray(x[c * BPC:(c + 1) * BPC])}
        m.update(w)
        in_maps.append(m)
    return bass_utils.run_bass_kernel_spmd(nc, in_maps,
                                           core_ids=list(range(NCORES)))


def kernel(**inputs):
    res = run(inputs)
    out = np.concatenate([res.results[c]["out"] for c in range(NCORES)], axis=0)
    return out.astype(np.float32)


# revision 16
# speedup vs baseline: 5750.6560x; 5750.6560x over previous
"""Trainium2 Bass kernel for a CoaT-style decoder block (ConvPosEnc +
FactorAttn w/ ConvRelPosEnc + FFN), data-parallel over batch on 8 cores.

Layout: activations channel-major [C(part), N(free)]. Matmuls are
weight-stationary (lhsT = W[cin, cout]) so outputs stay channel-major; the
kv einsum uses PE-transposed token-major tiles. Depthwise convs run on the
PE as per-tap diagonal matmuls over spatially shifted access patterns.
Matmul inputs are bf16 (fp32 PSUM accumulation); the residual stream lives
in-place in fp32 tiles; softmax and LN statistics are fp32.

SBUF tag plan (static allocation = sum over tags): the big [128,3136]
tiles share tag groups whose tenants have disjoint lifetimes:
  res{ct}  f32 : x^T -> x0 -> x0+attn -> out  (in-place residual)
  g1{ct} bf16  : xTb -> qT -> wfc1
  g2{ct} bf16  : x0b/x0s(in-place LN) -> attnT -> x0pb/y2(in-place LN)
  g3{ct} bf16  : sq -> ekT -> sq2 -> wfc2
  g4{ct} bf16  : vT  (g4_0 also hosts hdn [128,16,448])
  g5{0,1} bf16 : rbc/mbc -> wqkv01/23 -> ektok/vtok -> rbc2/mbc2
"""

import numpy as np
import ml_dtypes

import concourse.bass as bass
import concourse.bacc as bacc
import concourse.tile as tile
import concourse.mybir as mybir
from concourse import bass_utils

F32 = mybir.dt.float32
BF16 = mybir.dt.bfloat16
AF = mybir.ActivationFunctionType
OP = mybir.AluOpType
AX = mybir.AxisListType

B, NTOK, C = 16, 3136, 512
HH = WW = 56
NHEADS, CHD = 8, 64
HID = 2048
NCORES = 8
BPC = B // NCORES          # images per core
CT = 4                     # 128-channel tiles in C
CHUNK = 448                # tokens per matmul psum chunk (8 image rows)
NCHUNK = NTOK // CHUNK     # 7
RPC = 8                    # image rows per chunk
JT = 25                    # 128-token blocks (last has 64)
EPS = 1e-6

bf16 = ml_dtypes.bfloat16


def _taps(k):
    """Center-first tap list (first matmul must cover the full psum chunk)."""
    p = k // 2
    out = [(0, 0)]
    for dy in range(-p, p + 1):
        for dx in range(-p, p + 1):
            if (dy, dx) != (0, 0):
                out.append((dy, dx))
    return out

TAPS3, TAPS5, TAPS7 = _taps(3), _taps(5), _taps(7)
CRPE_TAPS = [TAPS3, TAPS5, TAPS7, TAPS7]
CRPE_OFF = [0, 9, 34, 83]
CRPE_NTAP = 132


def _diag_pack(ntap, weight_cols):
    out = np.zeros((128, ntap, 128), np.float32)
    idx = np.arange(128)
    for t in range(ntap):
        out[idx, t, idx] = weight_cols[t]
    return out.astype(bf16)


def _prep(inputs):
    g = lambda k: np.asarray(inputs[k], np.float32)
    x = g("x")
    qkv_w, proj_w, proj_b = g("qkv_w"), g("proj_w"), g("proj_b")
    fc1_w, fc1_b, fc2_w, fc2_b = g("fc1_w"), g("fc1_b"), g("fc2_w"), g("fc2_b")
    ln1_w, ln1_b, ln2_w, ln2_b = g("ln1_w"), g("ln1_b"), g("ln2_w"), g("ln2_b")
    cpe_w, cpe_b = g("cpe_w"), g("cpe_b")
    w3, b3, w5, b5, w7, b7 = g("w3"), g("b3"), g("w5"), g("b5"), g("w7"), g("b7")

    wqkv = (ln1_w[:, None] * qkv_w).astype(bf16)
    bqkv = ln1_b @ qkv_w
    wfc1 = (ln2_w[:, None] * fc1_w).astype(bf16)
    bfc1 = fc1_b + ln2_b @ fc1_w

    tiles = lambda b: np.ascontiguousarray(b.reshape(-1, 128).T)

    dcpe = np.concatenate(
        [_diag_pack(9, np.stack([cpe_w[ct * 128:(ct + 1) * 128, 0, dy + 1, dx + 1]
                                 for (dy, dx) in TAPS3]))
         for ct in range(CT)], axis=1)

    def crpe_cols(ct, taps):
        cols = []
        for (dy, dx) in taps:
            w = np.zeros(128, np.float32)
            for p in range(128):
                vch = ct * 128 + p
                if vch < 128:
                    if abs(dy) <= 1 and abs(dx) <= 1:
                        w[p] = w3[vch, 0, dy + 1, dx + 1]
                elif vch < 320:
                    if abs(dy) <= 2 and abs(dx) <= 2:
                        w[p] = w5[vch - 128, 0, dy + 2, dx + 2]
                else:
                    w[p] = w7[vch - 320, 0, dy + 3, dx + 3]
            cols.append(w)
        return np.stack(cols)

    dcrpe = np.concatenate(
        [_diag_pack(len(CRPE_TAPS[ct]), crpe_cols(ct, CRPE_TAPS[ct]))
         for ct in range(CT)], axis=1)

    w = {
        "wqkv": wqkv, "wproj": proj_w.astype(bf16),
        "wfc1": wfc1, "wfc2": fc2_w.astype(bf16),
        "bqkv": tiles(bqkv), "bproj": tiles(proj_b),
        "bfc1": tiles(bfc1), "bfc2": tiles(fc2_b),
        "bcpe": tiles(cpe_b), "bcrpe": tiles(np.concatenate([b3, b5, b7])),
        "dcpe": dcpe, "dcrpe": dcrpe,
        "crpw": np.concatenate([crpe_cols(ct, CRPE_TAPS[ct]).T
                                for ct in range(CT)], axis=1).astype(np.float32),
        "id_f32": np.eye(128, dtype=np.float32),
        "id_bf": np.eye(128, dtype=np.float32).astype(bf16),
        "ones_col": np.ones((128, 1), bf16),
        "ones_row": np.ones((1, 128), bf16),
    }
    return x, w


WEIGHT_SPECS = [
    ("wqkv", [C, 3 * C], BF16), ("wproj", [C, C], BF16),
    ("wfc1", [C, HID], BF16), ("wfc2", [HID, C], BF16),
    ("bqkv", [128, 12], F32), ("bproj", [128, 4], F32),
    ("bfc1", [128, 16], F32), ("bfc2", [128, 4], F32),
    ("bcpe", [128, 4], F32), ("bcrpe", [128, 4], F32),
    ("dcpe", [128, 36, 128], BF16), ("dcrpe", [128, CRPE_NTAP, 128], BF16),
    ("crpw", [128, CRPE_NTAP], F32),
    ("id_f32", [128, 128], F32), ("id_bf", [128, 128], BF16),
    ("ones_col", [128, 1], BF16), ("ones_row", [1, 128], BF16),
]

# token groups of four 128-blocks (used by transposes); last group is [24]
JGROUPS = [list(range(4 * g, 4 * g + 4)) for g in range(6)] + [[24]]
JW = lambda j: 128 if j < 24 else 64


class Builder:
    def __init__(self, nc, tc, aps, debug):
        self.nc, self.tc, self.aps, self.debug = nc, tc, aps, debug
        self.pools = {}

    def pool(self, name, bufs, space="SBUF"):
        if name not in self.pools:
            self.pools[name] = self.tc.alloc_tile_pool(name=name, bufs=bufs,
                                                       space=space)
        return self.pools[name]

    def dma(self, out, in_):
        self.nc.sync.dma_start(out=out, in_=in_)

    def big(self, tag, dtype=BF16, shape=None):
        return self.pool("pbig", 1).tile(shape or [128, NTOK], dtype,
                                         name=tag, tag=tag)

    def load_weights(self):
        nc, aps = self.nc, self.aps
        pw = self.pool("pw", 1)
        W = {}
        for ci in range(CT):
            t = pw.tile([128, C], BF16, name=f"wproj{ci}", tag=f"wproj{ci}")
            self.dma(t, aps["wproj"][ci * 128:(ci + 1) * 128, :])
            W[f"wproj{ci}"] = t
        for nm in ["bqkv", "bproj", "bfc1", "bfc2", "bcpe", "bcrpe",
                   "crpw", "id_f32", "id_bf", "ones_col", "ones_row"]:
            t = pw.tile(list(aps[nm].shape), aps[nm].dtype, name=nm, tag=nm)
            self.dma(t, aps[nm])
            W[nm] = t
        eps = pw.tile([128, 1], F32, name="eps", tag="eps")
        nc.vector.memset(eps, EPS)
        W["eps"] = eps
        self.W = W

    def dump(self, name, tiles):
        if not self.debug or name not in self.aps:
            return
        for i, t in enumerate(tiles):
            self.dma(self.aps[name][i], t)

    # ---------- stages ----------
    def transpose_in(self, img):
        """x[img] -> res (f32 channel-major) and xTb (bf16 copy)."""
        nc, W = self.nc, self.W
        ptok = self.pool("ptok", 1)
        pst = self.pool("pst", 2, space="PSUM")
        res = [self.big(f"res{ct}", F32) for ct in range(CT)]
        xTb = [self.big(f"g1_{ct}") for ct in range(CT)]
        for g, js in enumerate(JGROUPS):
            xtok = []
            for j in js:
                t = ptok.tile([128, C], F32, name=f"xtok{j % 4}", tag=f"xtok{j % 4}")
                self.dma(t[:JW(j)], self.aps["x"][img, j * 128:j * 128 + JW(j), :])
                xtok.append((t, JW(j)))
            for ct in range(CT):
                ps = pst.tile([128, 512], F32, name="tpf", tag="tpf")
                for i, (t, rows) in enumerate(xtok):
                    nc.tensor.transpose(ps[:, i * 128:i * 128 + rows],
                                        t[:rows, ct * 128:(ct + 1) * 128],
                                        W["id_f32"][:rows, :rows])
                width = sum(r for _, r in xtok)
                nc.scalar.copy(out=res[ct][:, g * 512:g * 512 + width],
                               in_=ps[:, :width])
        for ct in range(CT):
            nc.vector.tensor_copy(out=xTb[ct], in_=res[ct])
        return res, xTb

    def conv(self, chunk, diag, tap_list, tap_off, src_view, ps):
        nc = self.nc
        psv = ps.rearrange("p (h w) -> p h w", h=RPC)
        r0 = chunk * RPC
        n = len(tap_list)
        for t, (dy, dx) in enumerate(tap_list):
            y0 = max(r0, -dy)
            y1 = min(r0 + RPC, HH - max(0, dy))
            x0 = max(0, -dx)
            x1 = WW - max(0, dx)
            nc.tensor.matmul(
                psv[:, y0 - r0:y1 - r0, x0:x1],
                diag[:, tap_off + t, :],
                src_view[:, y0 + dy:y1 + dy, x0 + dx:x1 + dx],
                start=(t == 0), stop=(t == n - 1), skip_group_check=True)

    def cpe(self, img, res, xTb):
        """res = res + dwconv3(xTb) + bias (in-place); x0b = bf16(res)."""
        nc, W = self.nc, self.W
        pdiag = self.pool("pdiag", 1)
        pmm = self.pool("pmm", 4, space="PSUM")
        dcpe = pdiag.tile([128, 36, 128], BF16, name="diag", tag="diag")
        self.dma(dcpe, self.aps["dcpe"])
        x0b = [self.big(f"g2_{ct}") for ct in range(CT)]
        for ct in range(CT):
            src = xTb[ct].rearrange("p (h w) -> p h w", h=HH)
            for chunk in range(NCHUNK):
                ps = pmm.tile([128, CHUNK], F32, name="mm", tag="mm")
                self.conv(chunk, dcpe, TAPS3, ct * 9, src, ps)
                sl = bass.ts(chunk, CHUNK)
                nc.vector.scalar_tensor_tensor(
                    out=res[ct][:, sl], in0=ps, scalar=W["bcpe"][:, ct:ct + 1],
                    in1=res[ct][:, sl], op0=OP.add, op1=OP.add)
            nc.vector.tensor_copy(out=x0b[ct], in_=res[ct])
        self.dump(f"x0T_{img}", res)
        return x0b

    def ln(self, img, xb, sq_tags, bc_tags):
        """Channel-major LN over xb (list of 4 bf16 tiles), applied IN-PLACE.
        xb becomes the normalized tensor (gamma/beta folded downstream)."""
        nc, W = self.nc, self.W
        psm = self.pool("psmall", 1)
        pstat = self.pool("pst", 2, space="PSUM")
        sq = [self.big(t) for t in sq_tags]
        for ct in range(CT):
            nc.scalar.square(out=sq[ct], in_=xb[ct])
        # per-token sums over channels -> [128, 25] token-tiled stats
        st = psm.tile([128, JT], F32, name="st", tag="st")
        s2t = psm.tile([128, JT], F32, name="s2t", tag="s2t")
        for dst, srcs in ((st, xb), (s2t, sq)):
            pst_cols = pstat.tile([128, 32], F32, name="stt", tag="tpf")
            for g, js in enumerate(JGROUPS):
                w = sum(JW(j) for j in js)
                ps = pstat.tile([1, 512], F32, name="srow", tag="tpf")
                for ct in range(CT):
                    nc.tensor.matmul(ps[:, :w], W["ones_col"],
                                     srcs[ct][:, g * 512:g * 512 + w],
                                     start=(ct == 0), stop=(ct == CT - 1))
                rowb = psm.tile([1, 512], F32, name="rowb", tag="rowb")
                nc.scalar.copy(out=rowb[:, :w], in_=ps[:, :w])
                for i, j in enumerate(js):
                    nc.tensor.transpose(pst_cols[:JW(j), j:j + 1],
                                        rowb[0:1, i * 128:i * 128 + JW(j)],
                                        W["id_f32"][0:1, 0:1])
            nc.vector.tensor_copy(out=dst, in_=pst_cols[:, :JT])
        ms = psm.tile([128, JT], F32, name="ms", tag="ms")
        var = psm.tile([128, JT], F32, name="var", tag="var")
        rstd = psm.tile([128, JT], F32, name="rstd", tag="rstd")
        mrs = psm.tile([128, JT], F32, name="mrs", tag="mrs")
        nc.vector.tensor_scalar_mul(out=ms, in0=st, scalar1=1.0 / C)
        nc.vector.tensor_scalar_mul(out=var, in0=s2t, scalar1=1.0 / C)
        nc.vector.tensor_mul(out=st, in0=ms, in1=ms)
        nc.vector.tensor_sub(out=var, in0=var, in1=st)
        nc.scalar.activation(out=var, in_=var, func=AF.Sqrt, bias=W["eps"],
                             scale=1.0)
        nc.vector.reciprocal(out=rstd, in_=var)
        nc.vector.tensor_mul(out=mrs, in0=ms, in1=rstd)
        # broadcast rstd/mrs along partitions: [128,25] -> row chunks -> K=1 mm
        rbc = self.big(bc_tags[0])
        mbc = self.big(bc_tags[1])
        for dst, src in ((rbc, rstd), (mbc, mrs)):
            for g, js in enumerate(JGROUPS):
                w = sum(JW(j) for j in js)
                psr = pstat.tile([1, 512], F32, name="srow", tag="tpf")
                off = 0
                for j in js:
                    nc.tensor.transpose(psr[0:1, off:off + JW(j)],
                                        src[:JW(j), j:j + 1],
                                        W["id_f32"][:JW(j), :JW(j)])
                    off += JW(j)
                rowb = psm.tile([1, 512], BF16, name="rowbb", tag="rowbb")
                nc.scalar.copy(out=rowb[:, :w], in_=psr[:, :w])
                psb = pstat.tile([128, 512], F32, name="bc", tag="tpf")
                nc.tensor.matmul(psb[:, :w], W["ones_row"], rowb[0:1, :w],
                                 start=True, stop=True)
                nc.scalar.copy(out=dst[:, g * 512:g * 512 + w], in_=psb[:, :w])
        # apply in place: xb = xb * rbc - mbc
        for ct in range(CT):
            nc.vector.tensor_mul(out=xb[ct], in0=xb[ct], in1=rbc)
            nc.vector.tensor_sub(out=xb[ct], in0=xb[ct], in1=mbc)
        return xb

    def qkv(self, img, x0s):
        nc, W = self.nc, self.W
        psm = self.pool("psmall", 1)
        pmm = self.pool("pmm", 4, space="PSUM")
        wq = [self.big("g5_0", BF16, [128, 2, 3 * C]),
              self.big("g5_1", BF16, [128, 2, 3 * C])]
        for ci in range(CT):
            self.dma(wq[ci // 2][:, ci % 2, :],
                     self.aps["wqkv"][ci * 128:(ci + 1) * 128, :])
        qT = [self.big(f"g1_{t}") for t in range(CT)]
        ekT = [self.big(f"g3_{t}") for t in range(CT)]
        vT = [self.big(f"g4_{t}") if t > 0 else
              self.big("g4_0", BF16, [128, 16, CHUNK]) for t in range(CT)]
        vT[0] = vT[0].rearrange("p a b -> p (a b)")[:, :NTOK]
        sep = [psm.tile([128, NCHUNK], F32, name=f"sep{t}", tag=f"sep{t}")
               for t in range(CT)]
        recip = [psm.tile([128, 1], F32, name=f"rec{t}", tag=f"rec{t}")
                 for t in range(CT)]
        for co in range(12):
            for chunk in range(NCHUNK):
                ps = pmm.tile([128, CHUNK], F32, name="mm", tag="mm")
                for ci in range(CT):
                    nc.tensor.matmul(ps, wq[ci // 2][:, ci % 2,
                                                     co * 128:(co + 1) * 128],
                                     x0s[ci][:, bass.ts(chunk, CHUNK)],
                                     start=(ci == 0), stop=(ci == CT - 1))
                bias = W["bqkv"][:, co:co + 1]
                sl = bass.ts(chunk, CHUNK)
                if co < 4:
                    nc.scalar.activation(out=qT[co][:, sl], in_=ps,
                                         func=AF.Identity, bias=bias, scale=1.0)
                elif co < 8:
                    t = co - 4
                    nc.scalar.activation(out=ekT[t][:, sl], in_=ps, func=AF.Exp,
                                         bias=bias, scale=1.0,
                                         accum_out=sep[t][:, chunk:chunk + 1])
                else:
                    nc.scalar.activation(out=vT[co - 8][:, sl], in_=ps,
                                         func=AF.Identity, bias=bias, scale=1.0)
        for t in range(CT):
            s = psm.tile([128, 1], F32, name=f"sume{t}", tag=f"sume{t}")
            nc.vector.tensor_reduce(out=s, in_=sep[t], axis=AX.X, op=OP.add)
            nc.vector.reciprocal(out=recip[t], in_=s)
        self.dump(f"qT_{img}", qT)
        self.dump(f"ekT_{img}", ekT)
        self.dump(f"vT_{img}", vT)
        return qT, ekT, vT, recip

    def kv(self, img, ekT, vT, recip):
        nc, W = self.nc, self.W
        psm = self.pool("psmall", 1)
        pst = self.pool("pst", 2, space="PSUM")
        kv = []
        scale = CHD ** -0.5
        for t in range(CT):
            ektok = self.big("g5_0", BF16, [128, JT, 128])
            vtok = self.big("g5_1", BF16, [128, JT, 128])
            for src, dst in ((ekT[t], ektok), (vT[t], vtok)):
                for g, js in enumerate(JGROUPS):
                    ps = pst.tile([128, 512], BF16, name="tpb", tag="tpb")
                    for i, j in enumerate(js):
                        nc.tensor.transpose(ps[:JW(j), i * 128:(i + 1) * 128],
                                            src[:, j * 128:j * 128 + JW(j)],
                                            W["id_bf"])
                    for i, j in enumerate(js):
                        nc.vector.tensor_copy(
                            out=dst[:JW(j), j, :],
                            in_=ps[:JW(j), i * 128:(i + 1) * 128])
            ps = pst.tile([128, CHD], F32, name="kvps", tag="tpf")
            for h in range(2):
                for j in range(JT):
                    nc.tensor.matmul(
                        ps[h * 64:h * 64 + 64, :],
                        ektok[:JW(j), j, h * 64:h * 64 + 64],
                        vtok[:JW(j), j, h * 64:h * 64 + 64],
                        start=(j == 0), stop=(j == JT - 1),
                        tile_position=(0, h * 64))
            kvt = psm.tile([128, CHD], BF16, name=f"kv{t}", tag=f"kv{t}")
            nc.vector.tensor_scalar(out=kvt, in0=ps, scalar1=recip[t],
                                    scalar2=scale, op0=OP.mult, op1=OP.mult)
            kv.append(kvt)
        self.dump(f"kv_{img}", kv)
        return kv

    DVE_CONV = ()

    def conv_dve(self, ct, chunk, wcol, src_view, acc):
        nc = self.nc
        accv = acc.rearrange("p (h w) -> p h w", h=RPC)
        r0 = chunk * RPC
        for t, (dy, dx) in enumerate(CRPE_TAPS[ct]):
            y0 = max(r0, -dy)
            y1 = min(r0 + RPC, HH - max(0, dy))
            x0 = max(0, -dx)
            x1 = WW - max(0, dx)
            s = wcol[:, CRPE_OFF[ct] + t:CRPE_OFF[ct] + t + 1]
            iv = src_view[:, y0 + dy:y1 + dy, x0 + dx:x1 + dx]
            ov = accv[:, y0 - r0:y1 - r0, x0:x1]
            if t == 0:
                nc.vector.tensor_scalar_mul(out=acc, in0=iv, scalar1=s)
            else:
                nc.vector.scalar_tensor_tensor(out=ov, in0=iv, scalar=s,
                                               in1=ov, op0=OP.mult, op1=OP.add)

    def attn(self, img, qT, vT, kv):
        nc, W = self.nc, self.W
        pdiag = self.pool("pdiag", 1)
        psm = self.pool("psmall", 1)
        pmm = self.pool("pmm", 4, space="PSUM")
        attnT = [self.big(f"g2_{t}") for t in range(CT)]
        for ct in range(CT):
            ntap = len(CRPE_TAPS[ct])
            use_dve = ct in self.DVE_CONV
            if not use_dve:
                diag = pdiag.tile([128, 49, 128], BF16, name="diag", tag="diag")
                self.dma(diag[:, :ntap, :],
                         self.aps["dcrpe"][:, CRPE_OFF[ct]:CRPE_OFF[ct] + ntap, :])
            src = vT[ct].rearrange("p (h w) -> p h w", h=HH)
            for chunk in range(NCHUNK):
                sl = bass.ts(chunk, CHUNK)
                tmp = psm.tile([128, CHUNK], BF16, name="tmp", tag="tmp")
                if use_dve:
                    acc = psm.tile([128, CHUNK], F32, name="dacc", tag="dacc")
                    self.conv_dve(ct, chunk, W["crpw"], src, acc)
                    ps = acc
                else:
                    ps = pmm.tile([128, CHUNK], F32, name="mm", tag="mm")
                    self.conv(chunk, diag, CRPE_TAPS[ct], 0, src, ps)
                nc.vector.scalar_tensor_tensor(
                    out=tmp, in0=ps, scalar=W["bcrpe"][:, ct:ct + 1],
                    in1=qT[ct][:, sl], op0=OP.add, op1=OP.mult)
                ps2 = pmm.tile([128, CHUNK], F32, name="mm", tag="mm")
                for h in range(2):
                    nc.tensor.matmul(ps2[h * 64:h * 64 + 64, :],
                                     kv[ct][h * 64:h * 64 + 64, :],
                                     qT[ct][h * 64:h * 64 + 64, sl],
                                     start=True, stop=True,
                                     tile_position=(h * 64, h * 64))
                nc.vector.tensor_add(out=attnT[ct][:, sl], in0=ps2, in1=tmp)
        self.dump(f"attnT_{img}", attnT)
        return attnT

    def proj(self, img, attnT, res):
        nc, W = self.nc, self.W
        pmm = self.pool("pmm", 4, space="PSUM")
        for co in range(CT):
            for chunk in range(NCHUNK):
                ps = pmm.tile([128, CHUNK], F32, name="mm", tag="mm")
                for ci in range(CT):
                    nc.tensor.matmul(ps,
                                     W[f"wproj{ci}"][:, co * 128:(co + 1) * 128],
                                     attnT[ci][:, bass.ts(chunk, CHUNK)],
                                     start=(ci == 0), stop=(ci == CT - 1))
                sl = bass.ts(chunk, CHUNK)
                nc.vector.scalar_tensor_tensor(
                    out=res[co][:, sl], in0=ps, scalar=W["bproj"][:, co:co + 1],
                    in1=res[co][:, sl], op0=OP.add, op1=OP.add)
        x0pb = [self.big(f"g2_{t}") for t in range(CT)]
        for co in range(CT):
            nc.vector.tensor_copy(out=x0pb[co], in_=res[co])
        self.dump(f"x0pT_{img}", res)
        return x0pb

    def ffn(self, img, y2, res):
        nc, W = self.nc, self.W
        pmm = self.pool("pmm", 4, space="PSUM")
        wfc1 = [self.big(f"g1_{ci}", BF16, [128, HID]) for ci in range(CT)]
        wfc2 = [self.big(f"g3_{kt}", BF16, [128, 4, C]) for kt in range(CT)]
        for ci in range(CT):
            self.dma(wfc1[ci], self.aps["wfc1"][ci * 128:(ci + 1) * 128, :])
        for kt in range(16):
            self.dma(wfc2[kt // 4][:, kt % 4, :],
                     self.aps["wfc2"][kt * 128:(kt + 1) * 128, :])
        for chunk in range(NCHUNK):
            sl = bass.ts(chunk, CHUNK)
            hdn_ab = [self.big("g4_0", BF16, [128, 8, CHUNK]),
                      self.big("g4_1", BF16, [128, 8, CHUNK])]
            hdn = lambda kt: hdn_ab[kt // 8][:, kt % 8, :]
            for ho in range(16):
                ps = pmm.tile([128, CHUNK], F32, name="mm", tag="mm")
                for ci in range(CT):
                    nc.tensor.matmul(ps, wfc1[ci][:, ho * 128:(ho + 1) * 128],
                                     y2[ci][:, sl],
                                     start=(ci == 0), stop=(ci == CT - 1))
                nc.scalar.activation(out=hdn(ho), in_=ps, func=AF.Gelu,
                                     bias=W["bfc1"][:, ho:ho + 1], scale=1.0)
            for co in range(CT):
                ps = pmm.tile([128, CHUNK], F32, name="mm", tag="mm")
                for kt in range(16):
                    nc.tensor.matmul(ps,
                                     wfc2[kt // 4][:, kt % 4,
                                                   co * 128:(co + 1) * 128],
                                     hdn(kt),
                                     start=(kt == 0), stop=(kt == 15))
                nc.vector.scalar_tensor_tensor(
                    out=res[co][:, sl], in0=ps, scalar=W["bfc2"][:, co:co + 1],
                    in1=res[co][:, sl], op0=OP.add, op1=OP.add)
        self.dump(f"outT_{img}", res)

    def transpose_out(self, img, res):
        nc, W = self.nc, self.W
        ptok = self.pool("ptok", 1)
        pst = self.pool("pst", 2, space="PSUM")
        for j in range(JT):
            rows = JW(j)
            ps = pst.tile([128, 512], F32, name="tpf", tag="tpf")
            for ct in range(CT):
                nc.tensor.transpose(ps[:rows, ct * 128:(ct + 1) * 128],
                                    res[ct][:, j * 128:j * 128 + rows],
                                    W["id_f32"])
            t = ptok.tile([128, C], F32, name=f"xtok{j % 4}", tag=f"xtok{j % 4}")
            nc.scalar.copy(out=t[:rows], in_=ps[:rows])
            self.dma(self.aps["out"][img, j * 128:j * 128 + rows, :], t[:rows])

    def image(self, img):
        res, xTb = self.transpose_in(img)
        x0b = self.cpe(img, res, xTb)
        x0s = self.ln(img, x0b, [f"g3_{t}" for t in range(CT)],
                      ["g5_0", "g5_1"])
        self.dump(f"x0s_{img}", x0s)
        qT, ekT, vT, recip = self.qkv(img, x0s)
        kv = self.kv(img, ekT, vT, recip)
        attnT = self.attn(img, qT, vT, kv)
        x0pb = self.proj(img, attnT, res)
        y2 = self.ln(img, x0pb, [f"g3_{t}" for t in range(CT)],
                     ["g5_0", "g5_1"])
        self.dump(f"y2_{img}", y2)
        self.ffn(img, y2, res)
        self.transpose_out(img, res)

    def build(self):
        self.load_weights()
        for img in range(BPC):
            self.image(img)
        for p in reversed(list(self.pools.values())):
            p.release()


DEBUG_TENSORS = []
for img in range(BPC):
    DEBUG_TENSORS += [
        (f"x0T_{img}", F32), (f"x0s_{img}", BF16), (f"qT_{img}", BF16),
        (f"ekT_{img}", BF16), (f"vT_{img}", BF16), (f"attnT_{img}", BF16),
        (f"x0pT_{img}", F32), (f"y2_{img}", BF16), (f"outT_{img}", F32),
    ]


def build_nc(debug=False):
    nc = bacc.Bacc("TRN2", target_bir_lowering=False, debug=False,
                   num_devices=NCORES)
    aps = {}
    aps["x"] = nc.dram_tensor("x", [BPC, NTOK, C], F32, kind="ExternalInput").ap()
    for name, shape, dt in WEIGHT_SPECS:
        aps[name] = nc.dram_tensor(name, shape, dt, kind="ExternalInput").ap()
    aps["out"] = nc.dram_tensor("out", [BPC, NTOK, C], F32,
                                kind="ExternalOutput").ap()
    if debug:
        for name, dt in DEBUG_TENSORS:
            aps[name] = nc.dram_tensor(name, [CT, 128, NTOK], dt,
                                       kind="ExternalOutput").ap()
        aps["kv_0"] = nc.dram_tensor("kv_0", [CT, 128, CHD], BF16,
                                     kind="ExternalOutput").ap()
    with tile.TileContext(nc) as tc:
        Builder(nc, tc, aps, debug).build()
    nc.compile()
    return nc


_CACHE = {}


def run(inputs, debug=False):
    x, w = _prep(inputs)
    key = "dbg" if debug else "plain"
    if key not in _CACHE:
        _CACHE[key] = build_nc(debug)
    nc = _CACHE[key]
    in_maps = []
    for c in range(NCORES):
        m = {"x": np.ascontiguousarray(x[c * BPC:(c + 1) * BPC])}
        m.update(w)
        in_maps.append(m)
    return bass_utils.run_bass_kernel_spmd(nc, in_maps,
                                           core_ids=list(range(NCORES)))


def kernel(**inputs):
    res = run(inputs)
    out = np.concatenate([res.results[c]["out"] for c in range(NCORES)], axis=0)
    return out.astype(np.float32)


# revision 22
# speedup vs baseline: 5950.1461x; 1.0347x over previous
"""Trainium2 Bass kernel for a CoaT-style decoder block (ConvPosEnc +
FactorAttn w/ ConvRelPosEnc + FFN), data-parallel over batch on 8 cores.

Layout: activations channel-major [C(part), N(free)]. Matmuls are
weight-stationary (lhsT = W[cin, cout]) so outputs stay channel-major; the
kv einsum uses PE-transposed token-major tiles. Depthwise convs run on the
PE as per-tap diagonal matmuls over spatially shifted access patterns.
Matmul inputs are bf16 (fp32 PSUM accumulation); the residual stream lives
in-place in fp32 tiles; softmax and LN statistics are fp32.

SBUF tag plan (static allocation = sum over tags): the big [128,3136]
tiles share tag groups whose tenants have disjoint lifetimes:
  res{ct}  f32 : x^T -> x0 -> x0+attn -> out  (in-place residual)
  g1{ct} bf16  : xTb -> qT -> wfc1
  g2{ct} bf16  : x0b/x0s(in-place LN) -> attnT -> x0pb/y2(in-place LN)
  g3{ct} bf16  : sq -> ekT -> sq2 -> wfc2
  g4{ct} bf16  : vT  (g4_0 also hosts hdn [128,16,448])
  g5{0,1} bf16 : rbc/mbc -> wqkv01/23 -> ektok/vtok -> rbc2/mbc2
"""

import numpy as np
import ml_dtypes

import concourse.bass as bass
import concourse.bacc as bacc
import concourse.tile as tile
import concourse.mybir as mybir
from concourse import bass_utils

F32 = mybir.dt.float32
BF16 = mybir.dt.bfloat16
AF = mybir.ActivationFunctionType
OP = mybir.AluOpType
AX = mybir.AxisListType

B, NTOK, C = 16, 3136, 512
HH = WW = 56
NHEADS, CHD = 8, 64
HID = 2048
NCORES = 8
BPC = B // NCORES          # images per core
CT = 4                     # 128-channel tiles in C
CHUNK = 448                # tokens per matmul psum chunk (8 image rows)
NCHUNK = NTOK // CHUNK     # 7
RPC = 8                    # image rows per chunk
JT = 25                    # 128-token blocks (last has 64)
EPS = 1e-6

bf16 = ml_dtypes.bfloat16


def _taps(k):
    """Center-first tap list (first matmul must cover the full psum chunk)."""
    p = k // 2
    out = [(0, 0)]
    for dy in range(-p, p + 1):
        for dx in range(-p, p + 1):
            if (dy, dx) != (0, 0):
                out.append((dy, dx))
    return out

TAPS3, TAPS5, TAPS7 = _taps(3), _taps(5), _taps(7)
CRPE_TAPS = [TAPS3, TAPS5, TAPS7, TAPS7]
CRPE_OFF = [0, 9, 34, 83]
CRPE_NTAP = 132


def _diag_pack(ntap, weight_cols):
    out = np.zeros((128, ntap, 128), np.float32)
    idx = np.arange(128)
    for t in range(ntap):
        out[idx, t, idx] = weight_cols[t]
    return out.astype(bf16)


def _prep(inputs):
    g = lambda k: np.asarray(inputs[k], np.float32)
    x = g("x")
    qkv_w, proj_w, proj_b = g("qkv_w"), g("proj_w"), g("proj_b")
    fc1_w, fc1_b, fc2_w, fc2_b = g("fc1_w"), g("fc1_b"), g("fc2_w"), g("fc2_b")
    ln1_w, ln1_b, ln2_w, ln2_b = g("ln1_w"), g("ln1_b"), g("ln2_w"), g("ln2_b")
    cpe_w, cpe_b = g("cpe_w"), g("cpe_b")
    w3, b3, w5, b5, w7, b7 = g("w3"), g("b3"), g("w5"), g("b5"), g("w7"), g("b7")

    wqkv = (ln1_w[:, None] * qkv_w).astype(bf16)
    bqkv = ln1_b @ qkv_w
    wfc1 = (ln2_w[:, None] * fc1_w).astype(bf16)
    bfc1 = fc1_b + ln2_b @ fc1_w

    tiles = lambda b: np.ascontiguousarray(b.reshape(-1, 128).T)

    dcpe = np.concatenate(
        [_diag_pack(9, np.stack([cpe_w[ct * 128:(ct + 1) * 128, 0, dy + 1, dx + 1]
                                 for (dy, dx) in TAPS3]))
         for ct in range(CT)], axis=1)

    def crpe_cols(ct, taps):
        cols = []
        for (dy, dx) in taps:
            w = np.zeros(128, np.float32)
            for p in range(128):
                vch = ct * 128 + p
                if vch < 128:
                    if abs(dy) <= 1 and abs(dx) <= 1:
                        w[p] = w3[vch, 0, dy + 1, dx + 1]
                elif vch < 320:
                    if abs(dy) <= 2 and abs(dx) <= 2:
                        w[p] = w5[vch - 128, 0, dy + 2, dx + 2]
                else:
                    w[p] = w7[vch - 320, 0, dy + 3, dx + 3]
            cols.append(w)
        return np.stack(cols)

    dcrpe = np.concatenate(
        [_diag_pack(len(CRPE_TAPS[ct]), crpe_cols(ct, CRPE_TAPS[ct]))
         for ct in range(CT)], axis=1)

    w = {
        "wqkv": wqkv, "wproj": proj_w.astype(bf16),
        "wfc1": wfc1, "wfc2": fc2_w.astype(bf16),
        "bqkv": tiles(bqkv), "bproj": tiles(proj_b),
        "bfc1": tiles(bfc1), "bfc2": tiles(fc2_b),
        "bcpe": tiles(cpe_b), "bcrpe": tiles(np.concatenate([b3, b5, b7])),
        "dcpe": dcpe, "dcrpe": dcrpe,
        "crpw": np.concatenate([crpe_cols(ct, CRPE_TAPS[ct]).T
                                for ct in range(CT)], axis=1).astype(np.float32),
        "id_f32": np.eye(128, dtype=np.float32),
        "id_bf": np.eye(128, dtype=np.float32).astype(bf16),
        "ones_col": np.ones((128, 1), bf16),
        "ones_row": np.ones((1, 128), bf16),
    }
    return x, w


WEIGHT_SPECS = [
    ("wqkv", [C, 3 * C], BF16), ("wproj", [C, C], BF16),
    ("wfc1", [C, HID], BF16), ("wfc2", [HID, C], BF16),
    ("bqkv", [128, 12], F32), ("bproj", [128, 4], F32),
    ("bfc1", [128, 16], F32), ("bfc2", [128, 4], F32),
    ("bcpe", [128, 4], F32), ("bcrpe", [128, 4], F32),
    ("dcpe", [128, 36, 128], BF16), ("dcrpe", [128, CRPE_NTAP, 128], BF16),
    ("crpw", [128, CRPE_NTAP], F32),
    ("id_f32", [128, 128], F32), ("id_bf", [128, 128], BF16),
    ("ones_col", [128, 1], BF16), ("ones_row", [1, 128], BF16),
]

# token groups of four 128-blocks (used by transposes); last group is [24]
JGROUPS = [list(range(4 * g, 4 * g + 4)) for g in range(6)] + [[24]]
JW = lambda j: 128 if j < 24 else 64


class Builder:
    def __init__(self, nc, tc, aps, debug):
        self.nc, self.tc, self.aps, self.debug = nc, tc, aps, debug
        self.pools = {}

    def pool(self, name, bufs, space="SBUF"):
        if name not in self.pools:
            self.pools[name] = self.tc.alloc_tile_pool(name=name, bufs=bufs,
                                                       space=space)
        return self.pools[name]

    def dma(self, out, in_):
        self.nc.sync.dma_start(out=out, in_=in_)

    def big(self, tag, dtype=BF16, shape=None):
        return self.pool("pbig", 1).tile(shape or [128, NTOK], dtype,
                                         name=tag, tag=tag)

    def load_weights(self):
        nc, aps = self.nc, self.aps
        pw = self.pool("pw", 1)
        W = {}
        for ci in range(CT):
            t = pw.tile([128, C], BF16, name=f"wproj{ci}", tag=f"wproj{ci}")
            self.dma(t, aps["wproj"][ci * 128:(ci + 1) * 128, :])
            W[f"wproj{ci}"] = t
        for nm in ["bqkv", "bproj", "bfc1", "bfc2", "bcpe", "bcrpe",
                   "crpw", "id_f32", "id_bf", "ones_col", "ones_row"]:
            t = pw.tile(list(aps[nm].shape), aps[nm].dtype, name=nm, tag=nm)
            self.dma(t, aps[nm])
            W[nm] = t
        eps = pw.tile([128, 1], F32, name="eps", tag="eps")
        nc.vector.memset(eps, EPS)
        W["eps"] = eps
        self.W = W

    def dump(self, name, tiles):
        if not self.debug or name not in self.aps:
            return
        for i, t in enumerate(tiles):
            self.dma(self.aps[name][i], t)

    # ---------- stages ----------
    def transpose_in(self, img):
        """x[img] -> res (f32 channel-major) and xTb (bf16 copy)."""
        nc, W = self.nc, self.W
        ptok = self.pool("ptok", 1)
        pst = self.pool("pst", 2, space="PSUM")
        res = [self.big(f"res{ct}", F32) for ct in range(CT)]
        xTb = [self.big(f"g1_{ct}") for ct in range(CT)]
        for g, js in enumerate(JGROUPS):
            xtok = []
            for j in js:
                t = ptok.tile([128, C], F32, name=f"xtok{j % 6}", tag=f"xtok{j % 6}")
                self.dma(t[:JW(j)], self.aps["x"][img, j * 128:j * 128 + JW(j), :])
                xtok.append((t, JW(j)))
            for ct in range(CT):
                ps = pst.tile([128, 512], F32, name="tpf", tag="tpf", bufs=3)
                for i, (t, rows) in enumerate(xtok):
                    nc.tensor.transpose(ps[:, i * 128:i * 128 + rows],
                                        t[:rows, ct * 128:(ct + 1) * 128],
                                        W["id_f32"][:rows, :rows])
                width = sum(r for _, r in xtok)
                nc.scalar.copy(out=res[ct][:, g * 512:g * 512 + width],
                               in_=ps[:, :width])
        for ct in range(CT):
            nc.vector.tensor_copy(out=xTb[ct], in_=res[ct])
        return res, xTb

    def conv(self, chunk, diag, tap_list, tap_off, src_view, ps):
        nc = self.nc
        psv = ps.rearrange("p (h w) -> p h w", h=RPC)
        r0 = chunk * RPC
        n = len(tap_list)
        for t, (dy, dx) in enumerate(tap_list):
            y0 = max(r0, -dy)
            y1 = min(r0 + RPC, HH - max(0, dy))
            x0 = max(0, -dx)
            x1 = WW - max(0, dx)
            nc.tensor.matmul(
                psv[:, y0 - r0:y1 - r0, x0:x1],
                diag[:, tap_off + t, :],
                src_view[:, y0 + dy:y1 + dy, x0 + dx:x1 + dx],
                start=(t == 0), stop=(t == n - 1), skip_group_check=True)

    def cpe(self, img, res, xTb):
        """res = res + dwconv3(xTb) + bias (in-place); x0b = bf16(res)."""
        nc, W = self.nc, self.W
        pdiag = self.pool("pdiag", 1)
        pmm = self.pool("pmm", 3, space="PSUM")
        dcpe = pdiag.tile([128, 36, 128], BF16, name="diag", tag="diag")
        self.dma(dcpe, self.aps["dcpe"])
        x0b = [self.big(f"g2_{ct}") for ct in range(CT)]
        for ct in range(CT):
            src = xTb[ct].rearrange("p (h w) -> p h w", h=HH)
            for chunk in range(NCHUNK):
                ps = pmm.tile([128, CHUNK], F32, name="mm", tag="mm")
                self.conv(chunk, dcpe, TAPS3, ct * 9, src, ps)
                sl = bass.ts(chunk, CHUNK)
                nc.vector.scalar_tensor_tensor(
                    out=res[ct][:, sl], in0=ps, scalar=W["bcpe"][:, ct:ct + 1],
                    in1=res[ct][:, sl], op0=OP.add, op1=OP.add)
                nc.vector.tensor_copy(out=x0b[ct][:, sl], in_=res[ct][:, sl])
        self.dump(f"x0T_{img}", res)
        return x0b

    def ln(self, img, xb, sq_tags, bc_tags):
        """Channel-major LN over xb (list of 4 bf16 tiles), applied IN-PLACE.
        xb becomes the normalized tensor (gamma/beta folded downstream)."""
        nc, W = self.nc, self.W
        psm = self.pool("psmall", 1)
        pstat = self.pool("pst", 2, space="PSUM")
        sq = [self.big(t) for t in sq_tags]
        for ct in range(CT):
            nc.scalar.square(out=sq[ct], in_=xb[ct])
        # per-token sums over channels -> [128, 25] token-tiled stats
        st = psm.tile([128, JT], F32, name="st", tag="st")
        s2t = psm.tile([128, JT], F32, name="s2t", tag="s2t")
        for dst, srcs in ((st, xb), (s2t, sq)):
            pst_cols = pstat.tile([128, 32], F32, name="stt", tag="tpf", bufs=3)
            for g, js in enumerate(JGROUPS):
                w = sum(JW(j) for j in js)
                ps = pstat.tile([1, 512], F32, name="srow", tag="tpf", bufs=3)
                for ct in range(CT):
                    nc.tensor.matmul(ps[:, :w], W["ones_col"],
                                     srcs[ct][:, g * 512:g * 512 + w],
                                     start=(ct == 0), stop=(ct == CT - 1))
                rowb = psm.tile([1, 512], F32, name="rowb", tag="rowb")
                nc.scalar.copy(out=rowb[:, :w], in_=ps[:, :w])
                for i, j in enumerate(js):
                    nc.tensor.transpose(pst_cols[:JW(j), j:j + 1],
                                        rowb[0:1, i * 128:i * 128 + JW(j)],
                                        W["id_f32"][0:1, 0:1])
            nc.vector.tensor_copy(out=dst, in_=pst_cols[:, :JT])
        ms = psm.tile([128, JT], F32, name="ms", tag="ms")
        var = psm.tile([128, JT], F32, name="var", tag="var")
        rstd = psm.tile([128, JT], F32, name="rstd", tag="rstd")
        mrs = psm.tile([128, JT], F32, name="mrs", tag="mrs")
        nc.vector.tensor_scalar_mul(out=ms, in0=st, scalar1=1.0 / C)
        nc.vector.tensor_scalar_mul(out=var, in0=s2t, scalar1=1.0 / C)
        nc.vector.tensor_mul(out=st, in0=ms, in1=ms)
        nc.vector.tensor_sub(out=var, in0=var, in1=st)
        nc.scalar.activation(out=var, in_=var, func=AF.Sqrt, bias=W["eps"],
                             scale=1.0)
        nc.vector.reciprocal(out=rstd, in_=var)
        nc.vector.tensor_mul(out=mrs, in0=ms, in1=rstd)
        # broadcast rstd/mrs along partitions: [128,25] -> row chunks -> K=1 mm
        rbc = self.big(bc_tags[0])
        mbc = self.big(bc_tags[1])
        for dst, src in ((rbc, rstd), (mbc, mrs)):
            for g, js in enumerate(JGROUPS):
                w = sum(JW(j) for j in js)
                psr = pstat.tile([1, 512], F32, name="srow", tag="tpf", bufs=3)
                off = 0
                for j in js:
                    nc.tensor.transpose(psr[0:1, off:off + JW(j)],
                                        src[:JW(j), j:j + 1],
                                        W["id_f32"][:JW(j), :JW(j)])
                    off += JW(j)
                rowb = psm.tile([1, 512], BF16, name="rowbb", tag="rowbb")
                nc.scalar.copy(out=rowb[:, :w], in_=psr[:, :w])
                psb = pstat.tile([128, 512], F32, name="bc", tag="tpf", bufs=3)
                nc.tensor.matmul(psb[:, :w], W["ones_row"], rowb[0:1, :w],
                                 start=True, stop=True)
                nc.scalar.copy(out=dst[:, g * 512:g * 512 + w], in_=psb[:, :w])
        # apply in place: xb = xb * rbc - mbc
        for ct in range(CT):
            nc.vector.tensor_mul(out=xb[ct], in0=xb[ct], in1=rbc)
            nc.vector.tensor_sub(out=xb[ct], in0=xb[ct], in1=mbc)
        return xb

    def qkv(self, img, x0s):
        nc, W = self.nc, self.W
        psm = self.pool("psmall", 1)
        pmm = self.pool("pmm", 3, space="PSUM")
        wq = [self.big("g5_0", BF16, [128, 2, 3 * C]),
              self.big("g5_1", BF16, [128, 2, 3 * C])]
        for ci in range(CT):
            self.dma(wq[ci // 2][:, ci % 2, :],
                     self.aps["wqkv"][ci * 128:(ci + 1) * 128, :])
        qT = [self.big(f"g1_{t}") for t in range(CT)]
        ekT = [self.big(f"g3_{t}") for t in range(CT)]
        vT = [self.big(f"g4_{t}") if t > 0 else
              self.big("g4_0", BF16, [128, 16, CHUNK]) for t in range(CT)]
        vT[0] = vT[0].rearrange("p a b -> p (a b)")[:, :NTOK]
        sep = [psm.tile([128, NCHUNK], F32, name=f"sep{t}", tag=f"sep{t}")
               for t in range(CT)]
        recip = [psm.tile([128, 1], F32, name=f"rec{t}", tag=f"rec{t}")
                 for t in range(CT)]
        for co in range(12):
            for chunk in range(NCHUNK):
                ps = pmm.tile([128, CHUNK], F32, name="mm", tag="mm")
                for ci in range(CT):
                    nc.tensor.matmul(ps, wq[ci // 2][:, ci % 2,
                                                     co * 128:(co + 1) * 128],
                                     x0s[ci][:, bass.ts(chunk, CHUNK)],
                                     start=(ci == 0), stop=(ci == CT - 1))
                bias = W["bqkv"][:, co:co + 1]
                sl = bass.ts(chunk, CHUNK)
                if co < 4:
                    nc.scalar.activation(out=qT[co][:, sl], in_=ps,
                                         func=AF.Identity, bias=bias, scale=1.0)
                elif co < 8:
                    t = co - 4
                    nc.scalar.activation(out=ekT[t][:, sl], in_=ps, func=AF.Exp,
                                         bias=bias, scale=1.0,
                                         accum_out=sep[t][:, chunk:chunk + 1])
                else:
                    nc.scalar.activation(out=vT[co - 8][:, sl], in_=ps,
                                         func=AF.Identity, bias=bias, scale=1.0)
        for t in range(CT):
            s = psm.tile([128, 1], F32, name=f"sume{t}", tag=f"sume{t}")
            nc.vector.tensor_reduce(out=s, in_=sep[t], axis=AX.X, op=OP.add)
            nc.vector.reciprocal(out=recip[t], in_=s)
        self.dump(f"qT_{img}", qT)
        self.dump(f"ekT_{img}", ekT)
        self.dump(f"vT_{img}", vT)
        return qT, ekT, vT, recip

    def kv(self, img, ekT, vT, recip):
        nc, W = self.nc, self.W
        psm = self.pool("psmall", 1)
        pst = self.pool("pst", 2, space="PSUM")
        kv = []
        scale = CHD ** -0.5
        for t in range(CT):
            ektok = self.big("g5_0", BF16, [128, JT, 128])
            vtok = self.big("g5_1", BF16, [128, JT, 128])
            for src, dst in ((ekT[t], ektok), (vT[t], vtok)):
                for g, js in enumerate(JGROUPS):
                    ps = pst.tile([128, 512], BF16, name="tpb", tag="tpb")
                    for i, j in enumerate(js):
                        nc.tensor.transpose(ps[:JW(j), i * 128:(i + 1) * 128],
                                            src[:, j * 128:j * 128 + JW(j)],
                                            W["id_bf"])
                    for i, j in enumerate(js):
                        nc.vector.tensor_copy(
                            out=dst[:JW(j), j, :],
                            in_=ps[:JW(j), i * 128:(i + 1) * 128])
            ps = pst.tile([128, CHD], F32, name="kvps", tag="tpf", bufs=3)
            for h in range(2):
                for j in range(JT):
                    nc.tensor.matmul(
                        ps[h * 64:h * 64 + 64, :],
                        ektok[:JW(j), j, h * 64:h * 64 + 64],
                        vtok[:JW(j), j, h * 64:h * 64 + 64],
                        start=(j == 0), stop=(j == JT - 1),
                        tile_position=(0, h * 64))
            kvt = psm.tile([128, CHD], BF16, name=f"kv{t}", tag=f"kv{t}")
            nc.vector.tensor_scalar(out=kvt, in0=ps, scalar1=recip[t],
                                    scalar2=scale, op0=OP.mult, op1=OP.mult)
            kv.append(kvt)
        self.dump(f"kv_{img}", kv)
        return kv

    DVE_CONV = ()

    def conv_dve(self, ct, chunk, wcol, src_view, acc):
        nc = self.nc
        accv = acc.rearrange("p (h w) -> p h w", h=RPC)
        r0 = chunk * RPC
        for t, (dy, dx) in enumerate(CRPE_TAPS[ct]):
            y0 = max(r0, -dy)
            y1 = min(r0 + RPC, HH - max(0, dy))
            x0 = max(0, -dx)
            x1 = WW - max(0, dx)
            s = wcol[:, CRPE_OFF[ct] + t:CRPE_OFF[ct] + t + 1]
            iv = src_view[:, y0 + dy:y1 + dy, x0 + dx:x1 + dx]
            ov = accv[:, y0 - r0:y1 - r0, x0:x1]
            if t == 0:
                nc.vector.tensor_scalar_mul(out=acc, in0=iv, scalar1=s)
            else:
                nc.vector.scalar_tensor_tensor(out=ov, in0=iv, scalar=s,
                                               in1=ov, op0=OP.mult, op1=OP.add)

    def attn(self, img, qT, vT, kv):
        nc, W = self.nc, self.W
        pdiag = self.pool("pdiag", 1)
        psm = self.pool("psmall", 1)
        pmm = self.pool("pmm", 3, space="PSUM")
        attnT = [self.big(f"g2_{t}") for t in range(CT)]
        for ct in range(CT):
            ntap = len(CRPE_TAPS[ct])
            use_dve = ct in self.DVE_CONV
            if not use_dve:
                diag = pdiag.tile([128, 49, 128], BF16, name="diag", tag="diag")
                self.dma(diag[:, :ntap, :],
                         self.aps["dcrpe"][:, CRPE_OFF[ct]:CRPE_OFF[ct] + ntap, :])
            src = vT[ct].rearrange("p (h w) -> p h w", h=HH)
            for chunk in range(NCHUNK):
                sl = bass.ts(chunk, CHUNK)
                tmp = psm.tile([128, CHUNK], BF16, name="tmp", tag="tmp")
                if use_dve:
                    acc = psm.tile([128, CHUNK], F32, name="dacc", tag="dacc")
                    self.conv_dve(ct, chunk, W["crpw"], src, acc)
                    ps = acc
                else:
                    ps = pmm.tile([128, CHUNK], F32, name="mm", tag="mm")
                    self.conv(chunk, diag, CRPE_TAPS[ct], 0, src, ps)
                nc.vector.scalar_tensor_tensor(
                    out=tmp, in0=ps, scalar=W["bcrpe"][:, ct:ct + 1],
                    in1=qT[ct][:, sl], op0=OP.add, op1=OP.mult)
                ps2 = pmm.tile([128, CHUNK], F32, name="mm", tag="mm")
                for h in range(2):
                    nc.tensor.matmul(ps2[h * 64:h * 64 + 64, :],
                                     kv[ct][h * 64:h * 64 + 64, :],
                                     qT[ct][h * 64:h * 64 + 64, sl],
                                     start=True, stop=True,
                                     tile_position=(h * 64, h * 64))
                nc.vector.tensor_add(out=attnT[ct][:, sl], in0=ps2, in1=tmp)
        self.dump(f"attnT_{img}", attnT)
        return attnT

    def proj(self, img, attnT, res):
        nc, W = self.nc, self.W
        pmm = self.pool("pmm", 3, space="PSUM")
        x0pb = [self.big(f"g2_{t}") for t in range(CT)]
        for co in range(CT):
            for chunk in range(NCHUNK):
                ps = pmm.tile([128, CHUNK], F32, name="mm", tag="mm")
                for ci in range(CT):
                    nc.tensor.matmul(ps,
                                     W[f"wproj{ci}"][:, co * 128:(co + 1) * 128],
                                     attnT[ci][:, bass.ts(chunk, CHUNK)],
                                     start=(ci == 0), stop=(ci == CT - 1))
                sl = bass.ts(chunk, CHUNK)
                nc.vector.scalar_tensor_tensor(
                    out=res[co][:, sl], in0=ps, scalar=W["bproj"][:, co:co + 1],
                    in1=res[co][:, sl], op0=OP.add, op1=OP.add)
                nc.vector.tensor_copy(out=x0pb[co][:, sl], in_=res[co][:, sl])
        self.dump(f"x0pT_{img}", res)
        return x0pb

    def ffn(self, img, y2, res):
        nc, W = self.nc, self.W
        pmm = self.pool("pmm", 3, space="PSUM")
        wfc1 = [self.big(f"g1_{ci}", BF16, [128, HID]) for ci in range(CT)]
        wfc2 = [self.big(f"g3_{kt}", BF16, [128, 4, C]) for kt in range(CT)]
        for ci in range(CT):
            self.dma(wfc1[ci], self.aps["wfc1"][ci * 128:(ci + 1) * 128, :])
        for kt in range(16):
            self.dma(wfc2[kt // 4][:, kt % 4, :],
                     self.aps["wfc2"][kt * 128:(kt + 1) * 128, :])
        for chunk in range(NCHUNK):
            sl = bass.ts(chunk, CHUNK)
            tags = ("g4_0", "g4_1") if chunk % 2 == 0 else ("g5_0", "g5_1")
            hdn_ab = [self.big(tags[0], BF16, [128, 8, CHUNK]),
                      self.big(tags[1], BF16, [128, 8, CHUNK])]
            hdn = lambda kt: hdn_ab[kt // 8][:, kt % 8, :]
            for ho in range(16):
                ps = pmm.tile([128, CHUNK], F32, name="mm", tag="mm")
                for ci in range(CT):
                    nc.tensor.matmul(ps, wfc1[ci][:, ho * 128:(ho + 1) * 128],
                                     y2[ci][:, sl],
                                     start=(ci == 0), stop=(ci == CT - 1))
                nc.scalar.activation(out=hdn(ho), in_=ps, func=AF.Gelu,
                                     bias=W["bfc1"][:, ho:ho + 1], scale=1.0)
            for co in range(CT):
                ps = pmm.tile([128, CHUNK], F32, name="mm", tag="mm")
                for kt in range(16):
                    nc.tensor.matmul(ps,
                                     wfc2[kt // 4][:, kt % 4,
                                                   co * 128:(co + 1) * 128],
                                     hdn(kt),
                                     start=(kt == 0), stop=(kt == 15))
                nc.vector.scalar_tensor_tensor(
                    out=res[co][:, sl], in0=ps, scalar=W["bfc2"][:, co:co + 1],
                    in1=res[co][:, sl], op0=OP.add, op1=OP.add)
        self.dump(f"outT_{img}", res)

    def transpose_out(self, img, res):
        nc, W = self.nc, self.W
        ptok = self.pool("ptok", 1)
        pst = self.pool("pst", 2, space="PSUM")
        for j in range(JT):
            rows = JW(j)
            ps = pst.tile([128, 512], F32, name="tpf", tag="tpf", bufs=3)
            for ct in range(CT):
                nc.tensor.transpose(ps[:rows, ct * 128:(ct + 1) * 128],
                                    res[ct][:, j * 128:j * 128 + rows],
                                    W["id_f32"])
            t = ptok.tile([128, C], F32, name=f"xtok{j % 4}", tag=f"xtok{j % 4}")
            nc.scalar.copy(out=t[:rows], in_=ps[:rows])
            self.dma(self.aps["out"][img, j * 128:j * 128 + rows, :], t[:rows])

    def image(self, img):
        res, xTb = self.transpose_in(img)
        x0b = self.cpe(img, res, xTb)
        x0s = self.ln(img, x0b, [f"g3_{t}" for t in range(CT)],
                      ["g5_0", "g5_1"])
        self.dump(f"x0s_{img}", x0s)
        qT, ekT, vT, recip = self.qkv(img, x0s)
        kv = self.kv(img, ekT, vT, recip)
        attnT = self.attn(img, qT, vT, kv)
        x0pb = self.proj(img, attnT, res)
        y2 = self.ln(img, x0pb, [f"g3_{t}" for t in range(CT)],
                     ["g5_0", "g5_1"])
        self.dump(f"y2_{img}", y2)
        self.ffn(img, y2, res)
        self.transpose_out(img, res)

    def build(self):
        self.load_weights()
        for img in range(BPC):
            self.image(img)
        for p in reversed(list(self.pools.values())):
            p.release()


DEBUG_TENSORS = []
for img in range(BPC):
    DEBUG_TENSORS += [
        (f"x0T_{img}", F32), (f"x0s_{img}", BF16), (f"qT_{img}", BF16),
        (f"ekT_{img}", BF16), (f"vT_{img}", BF16), (f"attnT_{img}", BF16),
        (f"x0pT_{img}", F32), (f"y2_{img}", BF16), (f"outT_{img}", F32),
    ]


def build_nc(debug=False):
    nc = bacc.Bacc("TRN2", target_bir_lowering=False, debug=False,
                   num_devices=NCORES)
    aps = {}
    aps["x"] = nc.dram_tensor("x", [BPC, NTOK, C], F32, kind="ExternalInput").ap()
    for name, shape, dt in WEIGHT_SPECS:
        aps[name] = nc.dram_tensor(name, shape, dt, kind="ExternalInput").ap()
    aps["out"] = nc.dram_tensor("out", [BPC, NTOK, C], F32,
                                kind="ExternalOutput").ap()
    if debug:
        for name, dt in DEBUG_TENSORS:
            aps[name] = nc.dram_tensor(name, [CT, 128, NTOK], dt,
                                       kind="ExternalOutput").ap()
        aps["kv_0"] = nc.dram_tensor("kv_0", [CT, 128, CHD], BF16,
                                     kind="ExternalOutput").ap()
    with tile.TileContext(nc) as tc:
        Builder(nc, tc, aps, debug).build()
    nc.compile()
    return nc


_CACHE = {}


def run(inputs, debug=False):
    x, w = _prep(inputs)
    key = "dbg" if debug else "plain"
    if key not in _CACHE:
        _CACHE[key] = build_nc(debug)
    nc = _CACHE[key]
    in_maps = []
    for c in range(NCORES):
        m = {"x": np.ascontiguousarray(x[c * BPC:(c + 1) * BPC])}
        m.update(w)
        in_maps.append(m)
    return bass_utils.run_bass_kernel_spmd(nc, in_maps,
                                           core_ids=list(range(NCORES)))


def kernel(**inputs):
    res = run(inputs)
    out = np.concatenate([res.results[c]["out"] for c in range(NCORES)], axis=0)
    return out.astype(np.float32)


# revision 23
# speedup vs baseline: 6020.2885x; 1.0118x over previous
"""Trainium2 Bass kernel for a CoaT-style decoder block (ConvPosEnc +
FactorAttn w/ ConvRelPosEnc + FFN), data-parallel over batch on 8 cores.

Layout: activations channel-major [C(part), N(free)]. Matmuls are
weight-stationary (lhsT = W[cin, cout]) so outputs stay channel-major; the
kv einsum uses PE-transposed token-major tiles. Depthwise convs run on the
PE as per-tap diagonal matmuls over spatially shifted access patterns.
Matmul inputs are bf16 (fp32 PSUM accumulation); the residual stream lives
in-place in fp32 tiles; softmax and LN statistics are fp32.

SBUF tag plan (static allocation = sum over tags): the big [128,3136]
tiles share tag groups whose tenants have disjoint lifetimes:
  res{ct}  f32 : x^T -> x0 -> x0+attn -> out  (in-place residual)
  g1{ct} bf16  : xTb -> qT -> wfc1
  g2{ct} bf16  : x0b/x0s(in-place LN) -> attnT -> x0pb/y2(in-place LN)
  g3{ct} bf16  : sq -> ekT -> sq2 -> wfc2
  g4{ct} bf16  : vT  (g4_0 also hosts hdn [128,16,448])
  g5{0,1} bf16 : rbc/mbc -> wqkv01/23 -> ektok/vtok -> rbc2/mbc2
"""

import numpy as np
import ml_dtypes

import concourse.bass as bass
import concourse.bacc as bacc
import concourse.tile as tile
import concourse.mybir as mybir
from concourse import bass_utils

F32 = mybir.dt.float32
BF16 = mybir.dt.bfloat16
AF = mybir.ActivationFunctionType
OP = mybir.AluOpType
AX = mybir.AxisListType

B, NTOK, C = 16, 3136, 512
HH = WW = 56
NHEADS, CHD = 8, 64
HID = 2048
NCORES = 8
BPC = B // NCORES          # images per core
CT = 4                     # 128-channel tiles in C
CHUNK = 448                # tokens per matmul psum chunk (8 image rows)
NCHUNK = NTOK // CHUNK     # 7
RPC = 8                    # image rows per chunk
JT = 25                    # 128-token blocks (last has 64)
EPS = 1e-6

bf16 = ml_dtypes.bfloat16


def _taps(k):
    """Center-first tap list (first matmul must cover the full psum chunk)."""
    p = k // 2
    out = [(0, 0)]
    for dy in range(-p, p + 1):
        for dx in range(-p, p + 1):
            if (dy, dx) != (0, 0):
                out.append((dy, dx))
    return out

TAPS3, TAPS5, TAPS7 = _taps(3), _taps(5), _taps(7)
CRPE_TAPS = [TAPS3, TAPS5, TAPS7, TAPS7]
CRPE_OFF = [0, 9, 34, 83]
CRPE_NTAP = 132


def _diag_pack(ntap, weight_cols):
    out = np.zeros((128, ntap, 128), np.float32)
    idx = np.arange(128)
    for t in range(ntap):
        out[idx, t, idx] = weight_cols[t]
    return out.astype(bf16)


def _prep(inputs):
    g = lambda k: np.asarray(inputs[k], np.float32)
    x = g("x")
    qkv_w, proj_w, proj_b = g("qkv_w"), g("proj_w"), g("proj_b")
    fc1_w, fc1_b, fc2_w, fc2_b = g("fc1_w"), g("fc1_b"), g("fc2_w"), g("fc2_b")
    ln1_w, ln1_b, ln2_w, ln2_b = g("ln1_w"), g("ln1_b"), g("ln2_w"), g("ln2_b")
    cpe_w, cpe_b = g("cpe_w"), g("cpe_b")
    w3, b3, w5, b5, w7, b7 = g("w3"), g("b3"), g("w5"), g("b5"), g("w7"), g("b7")

    wqkv = (ln1_w[:, None] * qkv_w).astype(bf16)
    bqkv = ln1_b @ qkv_w
    wfc1 = (ln2_w[:, None] * fc1_w).astype(bf16)
    bfc1 = fc1_b + ln2_b @ fc1_w

    tiles = lambda b: np.ascontiguousarray(b.reshape(-1, 128).T)

    dcpe = np.concatenate(
        [_diag_pack(9, np.stack([cpe_w[ct * 128:(ct + 1) * 128, 0, dy + 1, dx + 1]
                                 for (dy, dx) in TAPS3]))
         for ct in range(CT)], axis=1)

    def crpe_cols(ct, taps):
        cols = []
        for (dy, dx) in taps:
            w = np.zeros(128, np.float32)
            for p in range(128):
                vch = ct * 128 + p
                if vch < 128:
                    if abs(dy) <= 1 and abs(dx) <= 1:
                        w[p] = w3[vch, 0, dy + 1, dx + 1]
                elif vch < 320:
                    if abs(dy) <= 2 and abs(dx) <= 2:
                        w[p] = w5[vch - 128, 0, dy + 2, dx + 2]
                else:
                    w[p] = w7[vch - 320, 0, dy + 3, dx + 3]
            cols.append(w)
        return np.stack(cols)

    dcrpe = np.concatenate(
        [_diag_pack(len(CRPE_TAPS[ct]), crpe_cols(ct, CRPE_TAPS[ct]))
         for ct in range(CT)], axis=1)

    w = {
        "wqkv": wqkv, "wproj": proj_w.astype(bf16),
        "wfc1": wfc1, "wfc2": fc2_w.astype(bf16),
        "bqkv": tiles(bqkv), "bproj": tiles(proj_b),
        "bfc1": tiles(bfc1), "bfc2": tiles(fc2_b),
        "bcpe": tiles(cpe_b), "bcrpe": tiles(np.concatenate([b3, b5, b7])),
        "dcpe": dcpe, "dcrpe": dcrpe,
        "crpw": np.concatenate([crpe_cols(ct, CRPE_TAPS[ct]).T
                                for ct in range(CT)], axis=1).astype(np.float32),
        "id_f32": np.eye(128, dtype=np.float32),
        "id_bf": np.eye(128, dtype=np.float32).astype(bf16),
        "ones_col": np.ones((128, 1), bf16),
        "ones_row": np.ones((1, 128), bf16),
    }
    return x, w


WEIGHT_SPECS = [
    ("wqkv", [C, 3 * C], BF16), ("wproj", [C, C], BF16),
    ("wfc1", [C, HID], BF16), ("wfc2", [HID, C], BF16),
    ("bqkv", [128, 12], F32), ("bproj", [128, 4], F32),
    ("bfc1", [128, 16], F32), ("bfc2", [128, 4], F32),
    ("bcpe", [128, 4], F32), ("bcrpe", [128, 4], F32),
    ("dcpe", [128, 36, 128], BF16), ("dcrpe", [128, CRPE_NTAP, 128], BF16),
    ("crpw", [128, CRPE_NTAP], F32),
    ("id_f32", [128, 128], F32), ("id_bf", [128, 128], BF16),
    ("ones_col", [128, 1], BF16), ("ones_row", [1, 128], BF16),
]

# token groups of four 128-blocks (used by transposes); last group is [24]
JGROUPS = [list(range(4 * g, 4 * g + 4)) for g in range(6)] + [[24]]
JW = lambda j: 128 if j < 24 else 64


class Builder:
    def __init__(self, nc, tc, aps, debug):
        self.nc, self.tc, self.aps, self.debug = nc, tc, aps, debug
        self.pools = {}

    def pool(self, name, bufs, space="SBUF"):
        if name not in self.pools:
            self.pools[name] = self.tc.alloc_tile_pool(name=name, bufs=bufs,
                                                       space=space)
        return self.pools[name]

    def dma(self, out, in_):
        self.nc.sync.dma_start(out=out, in_=in_)

    def big(self, tag, dtype=BF16, shape=None):
        return self.pool("pbig", 1).tile(shape or [128, NTOK], dtype,
                                         name=tag, tag=tag)

    def load_weights(self):
        nc, aps = self.nc, self.aps
        pw = self.pool("pw", 1)
        W = {}
        for ci in range(CT):
            t = pw.tile([128, C], BF16, name=f"wproj{ci}", tag=f"wproj{ci}")
            self.dma(t, aps["wproj"][ci * 128:(ci + 1) * 128, :])
            W[f"wproj{ci}"] = t
        for nm in ["bqkv", "bproj", "bfc1", "bfc2", "bcpe", "bcrpe",
                   "crpw", "id_f32", "id_bf", "ones_col", "ones_row"]:
            t = pw.tile(list(aps[nm].shape), aps[nm].dtype, name=nm, tag=nm)
            self.dma(t, aps[nm])
            W[nm] = t
        eps = pw.tile([128, 1], F32, name="eps", tag="eps")
        nc.vector.memset(eps, EPS)
        W["eps"] = eps
        self.W = W

    def dump(self, name, tiles):
        if not self.debug or name not in self.aps:
            return
        for i, t in enumerate(tiles):
            self.dma(self.aps[name][i], t)

    # ---------- stages ----------
    def transpose_in(self, img):
        """x[img] -> res (f32 channel-major) and xTb (bf16 copy)."""
        nc, W = self.nc, self.W
        ptok = self.pool("ptok", 1)
        pst = self.pool("pst", 2, space="PSUM")
        res = [self.big(f"res{ct}", F32) for ct in range(CT)]
        xTb = [self.big(f"g1_{ct}") for ct in range(CT)]
        for g, js in enumerate(JGROUPS):
            xtok = []
            for j in js:
                t = ptok.tile([128, C], F32, name=f"xtok{j % 6}", tag=f"xtok{j % 6}")
                self.dma(t[:JW(j)], self.aps["x"][img, j * 128:j * 128 + JW(j), :])
                xtok.append((t, JW(j)))
            for ct in range(CT):
                ps = pst.tile([128, 512], F32, name="tpf", tag="tpf", bufs=3)
                for i, (t, rows) in enumerate(xtok):
                    nc.tensor.transpose(ps[:, i * 128:i * 128 + rows],
                                        t[:rows, ct * 128:(ct + 1) * 128],
                                        W["id_f32"][:rows, :rows])
                width = sum(r for _, r in xtok)
                nc.scalar.copy(out=res[ct][:, g * 512:g * 512 + width],
                               in_=ps[:, :width])
        for ct in range(CT):
            nc.vector.tensor_copy(out=xTb[ct], in_=res[ct])
        return res, xTb

    def conv(self, chunk, diag, tap_list, tap_off, src_view, ps):
        nc = self.nc
        psv = ps.rearrange("p (h w) -> p h w", h=RPC)
        r0 = chunk * RPC
        n = len(tap_list)
        for t, (dy, dx) in enumerate(tap_list):
            y0 = max(r0, -dy)
            y1 = min(r0 + RPC, HH - max(0, dy))
            x0 = max(0, -dx)
            x1 = WW - max(0, dx)
            nc.tensor.matmul(
                psv[:, y0 - r0:y1 - r0, x0:x1],
                diag[:, tap_off + t, :],
                src_view[:, y0 + dy:y1 + dy, x0 + dx:x1 + dx],
                start=(t == 0), stop=(t == n - 1), skip_group_check=True)

    def cpe(self, img, res, xTb):
        """res = res + dwconv3(xTb) + bias (in-place); x0b = bf16(res)."""
        nc, W = self.nc, self.W
        pdiag = self.pool("pdiag", 1)
        pmm = self.pool("pmm", 3, space="PSUM")
        dcpe = pdiag.tile([128, 36, 128], BF16, name="diag", tag="diag")
        self.dma(dcpe, self.aps["dcpe"])
        x0b = [self.big(f"g2_{ct}") for ct in range(CT)]
        for ct in range(CT):
            src = xTb[ct].rearrange("p (h w) -> p h w", h=HH)
            for chunk in range(NCHUNK):
                ps = pmm.tile([128, CHUNK], F32, name="mm", tag="mm")
                self.conv(chunk, dcpe, TAPS3, ct * 9, src, ps)
                sl = bass.ts(chunk, CHUNK)
                nc.vector.scalar_tensor_tensor(
                    out=res[ct][:, sl], in0=ps, scalar=W["bcpe"][:, ct:ct + 1],
                    in1=res[ct][:, sl], op0=OP.add, op1=OP.add)
                nc.vector.tensor_copy(out=x0b[ct][:, sl], in_=res[ct][:, sl])
        self.dump(f"x0T_{img}", res)
        return x0b

    def ln(self, img, xb, sq_tags, bc_tags):
        """Channel-major LN over xb (list of 4 bf16 tiles), applied IN-PLACE.
        xb becomes the normalized tensor (gamma/beta folded downstream)."""
        nc, W = self.nc, self.W
        psm = self.pool("psmall", 1)
        pstat = self.pool("pst", 2, space="PSUM")
        sq = [self.big(t) for t in sq_tags]
        for ct in range(CT):
            nc.scalar.square(out=sq[ct], in_=xb[ct])
        # per-token sums over channels -> [128, 25] token-tiled stats
        st = psm.tile([128, JT], F32, name="st", tag="st")
        s2t = psm.tile([128, JT], F32, name="s2t", tag="s2t")
        for dst, srcs in ((st, xb), (s2t, sq)):
            pst_cols = pstat.tile([128, 32], F32, name="stt", tag="tpf", bufs=3)
            for g, js in enumerate(JGROUPS):
                w = sum(JW(j) for j in js)
                ps = pstat.tile([1, 512], F32, name="srow", tag="tpf", bufs=3)
                for ct in range(CT):
                    nc.tensor.matmul(ps[:, :w], W["ones_col"],
                                     srcs[ct][:, g * 512:g * 512 + w],
                                     start=(ct == 0), stop=(ct == CT - 1))
                rowb = psm.tile([1, 512], F32, name="rowb", tag="rowb")
                nc.scalar.copy(out=rowb[:, :w], in_=ps[:, :w])
                for i, j in enumerate(js):
                    nc.tensor.transpose(pst_cols[:JW(j), j:j + 1],
                                        rowb[0:1, i * 128:i * 128 + JW(j)],
                                        W["id_f32"][0:1, 0:1])
            nc.vector.tensor_copy(out=dst, in_=pst_cols[:, :JT])
        ms = psm.tile([128, JT], F32, name="ms", tag="ms")
        var = psm.tile([128, JT], F32, name="var", tag="var")
        rstd = psm.tile([128, JT], F32, name="rstd", tag="rstd")
        mrs = psm.tile([128, JT], F32, name="mrs", tag="mrs")
        nc.vector.tensor_scalar_mul(out=ms, in0=st, scalar1=1.0 / C)
        nc.vector.tensor_scalar_mul(out=var, in0=s2t, scalar1=1.0 / C)
        nc.vector.tensor_mul(out=st, in0=ms, in1=ms)
        nc.vector.tensor_sub(out=var, in0=var, in1=st)
        nc.scalar.activation(out=var, in_=var, func=AF.Sqrt, bias=W["eps"],
                             scale=1.0)
        nc.vector.reciprocal(out=rstd, in_=var)
        nc.vector.tensor_mul(out=mrs, in0=ms, in1=rstd)
        # broadcast rstd/mrs along partitions: [128,25] -> row chunks -> K=1 mm
        rbc = self.big(bc_tags[0])
        mbc = self.big(bc_tags[1])
        for dst, src in ((rbc, rstd), (mbc, mrs)):
            for g, js in enumerate(JGROUPS):
                w = sum(JW(j) for j in js)
                psr = pstat.tile([1, 512], F32, name="srow", tag="tpf", bufs=3)
                off = 0
                for j in js:
                    nc.tensor.transpose(psr[0:1, off:off + JW(j)],
                                        src[:JW(j), j:j + 1],
                                        W["id_f32"][:JW(j), :JW(j)])
                    off += JW(j)
                rowb = psm.tile([1, 512], BF16, name="rowbb", tag="rowbb")
                nc.scalar.copy(out=rowb[:, :w], in_=psr[:, :w])
                psb = pstat.tile([128, 512], F32, name="bc", tag="tpf", bufs=3)
                nc.tensor.matmul(psb[:, :w], W["ones_row"], rowb[0:1, :w],
                                 start=True, stop=True)
                nc.scalar.copy(out=dst[:, g * 512:g * 512 + w], in_=psb[:, :w])
        # apply in place: xb = xb * rbc - mbc
        for ct in range(CT):
            nc.vector.tensor_mul(out=xb[ct], in0=xb[ct], in1=rbc)
            nc.vector.tensor_sub(out=xb[ct], in0=xb[ct], in1=mbc)
        return xb

    def qkv(self, img, x0s):
        nc, W = self.nc, self.W
        psm = self.pool("psmall", 1)
        pmm = self.pool("pmm", 3, space="PSUM")
        wq = [self.big("g5_0", BF16, [128, 2, 3 * C]),
              self.big("g5_1", BF16, [128, 2, 3 * C])]
        for ci in range(CT):
            self.dma(wq[ci // 2][:, ci % 2, :],
                     self.aps["wqkv"][ci * 128:(ci + 1) * 128, :])
        qT = [self.big(f"g1_{t}") for t in range(CT)]
        ekT = [self.big(f"g3_{t}") for t in range(CT)]
        vT = [self.big(f"g4_{t}") if t > 0 else
              self.big("g4_0", BF16, [128, 16, CHUNK]) for t in range(CT)]
        vT[0] = vT[0].rearrange("p a b -> p (a b)")[:, :NTOK]
        sep = [psm.tile([128, NCHUNK], F32, name=f"sep{t}", tag=f"sep{t}")
               for t in range(CT)]
        recip = [psm.tile([128, 1], F32, name=f"rec{t}", tag=f"rec{t}")
                 for t in range(CT)]
        for co in range(12):
            for chunk in range(NCHUNK):
                ps = pmm.tile([128, CHUNK], F32, name="mm", tag="mm")
                for ci in range(CT):
                    nc.tensor.matmul(ps, wq[ci // 2][:, ci % 2,
                                                     co * 128:(co + 1) * 128],
                                     x0s[ci][:, bass.ts(chunk, CHUNK)],
                                     start=(ci == 0), stop=(ci == CT - 1))
                bias = W["bqkv"][:, co:co + 1]
                sl = bass.ts(chunk, CHUNK)
                if co < 4:
                    nc.scalar.activation(out=qT[co][:, sl], in_=ps,
                                         func=AF.Identity, bias=bias, scale=1.0)
                elif co < 8:
                    t = co - 4
                    nc.scalar.activation(out=ekT[t][:, sl], in_=ps, func=AF.Exp,
                                         bias=bias, scale=1.0,
                                         accum_out=sep[t][:, chunk:chunk + 1])
                else:
                    nc.scalar.activation(out=vT[co - 8][:, sl], in_=ps,
                                         func=AF.Identity, bias=bias, scale=1.0)
        for t in range(CT):
            s = psm.tile([128, 1], F32, name=f"sume{t}", tag=f"sume{t}")
            nc.vector.tensor_reduce(out=s, in_=sep[t], axis=AX.X, op=OP.add)
            nc.vector.reciprocal(out=recip[t], in_=s)
        self.dump(f"qT_{img}", qT)
        self.dump(f"ekT_{img}", ekT)
        self.dump(f"vT_{img}", vT)
        return qT, ekT, vT, recip

    def kv(self, img, ekT, vT, recip):
        nc, W = self.nc, self.W
        psm = self.pool("psmall", 1)
        pst = self.pool("pst", 2, space="PSUM")
        kv = []
        scale = CHD ** -0.5
        for t in range(CT):
            ektok = self.big("g5_0", BF16, [128, JT, 128])
            vtok = self.big("g5_1", BF16, [128, JT, 128])
            for src, dst in ((ekT[t], ektok), (vT[t], vtok)):
                for g, js in enumerate(JGROUPS):
                    ps = pst.tile([128, 512], BF16, name="tpb", tag="tpb")
                    for i, j in enumerate(js):
                        nc.tensor.transpose(ps[:JW(j), i * 128:(i + 1) * 128],
                                            src[:, j * 128:j * 128 + JW(j)],
                                            W["id_bf"])
                    if len(js) == 4:
                        nc.vector.tensor_copy(
                            out=dst[:, js[0]:js[0] + 4, :].rearrange(
                                "p a b -> p (a b)"),
                            in_=ps[:, :512])
                    else:
                        for i, j in enumerate(js):
                            nc.vector.tensor_copy(
                                out=dst[:JW(j), j, :],
                                in_=ps[:JW(j), i * 128:(i + 1) * 128])
            ps = pst.tile([128, CHD], F32, name="kvps", tag="tpf", bufs=3)
            for h in range(2):
                for j in range(JT):
                    nc.tensor.matmul(
                        ps[h * 64:h * 64 + 64, :],
                        ektok[:JW(j), j, h * 64:h * 64 + 64],
                        vtok[:JW(j), j, h * 64:h * 64 + 64],
                        start=(j == 0), stop=(j == JT - 1),
                        tile_position=(0, h * 64))
            kvt = psm.tile([128, CHD], BF16, name=f"kv{t}", tag=f"kv{t}")
            nc.vector.tensor_scalar(out=kvt, in0=ps, scalar1=recip[t],
                                    scalar2=scale, op0=OP.mult, op1=OP.mult)
            kv.append(kvt)
        self.dump(f"kv_{img}", kv)
        return kv

    DVE_CONV = ()

    def conv_dve(self, ct, chunk, wcol, src_view, acc):
        nc = self.nc
        accv = acc.rearrange("p (h w) -> p h w", h=RPC)
        r0 = chunk * RPC
        for t, (dy, dx) in enumerate(CRPE_TAPS[ct]):
            y0 = max(r0, -dy)
            y1 = min(r0 + RPC, HH - max(0, dy))
            x0 = max(0, -dx)
            x1 = WW - max(0, dx)
            s = wcol[:, CRPE_OFF[ct] + t:CRPE_OFF[ct] + t + 1]
            iv = src_view[:, y0 + dy:y1 + dy, x0 + dx:x1 + dx]
            ov = accv[:, y0 - r0:y1 - r0, x0:x1]
            if t == 0:
                nc.vector.tensor_scalar_mul(out=acc, in0=iv, scalar1=s)
            else:
                nc.vector.scalar_tensor_tensor(out=ov, in0=iv, scalar=s,
                                               in1=ov, op0=OP.mult, op1=OP.add)

    def attn(self, img, qT, vT, kv):
        nc, W = self.nc, self.W
        pdiag = self.pool("pdiag", 1)
        psm = self.pool("psmall", 1)
        pmm = self.pool("pmm", 3, space="PSUM")
        attnT = [self.big(f"g2_{t}") for t in range(CT)]
        for ct in range(CT):
            ntap = len(CRPE_TAPS[ct])
            use_dve = ct in self.DVE_CONV
            if not use_dve:
                diag = pdiag.tile([128, 49, 128], BF16, name="diag", tag="diag")
                self.dma(diag[:, :ntap, :],
                         self.aps["dcrpe"][:, CRPE_OFF[ct]:CRPE_OFF[ct] + ntap, :])
            src = vT[ct].rearrange("p (h w) -> p h w", h=HH)
            for chunk in range(NCHUNK):
                sl = bass.ts(chunk, CHUNK)
                tmp = psm.tile([128, CHUNK], BF16, name="tmp", tag="tmp")
                if use_dve:
                    acc = psm.tile([128, CHUNK], F32, name="dacc", tag="dacc")
                    self.conv_dve(ct, chunk, W["crpw"], src, acc)
                    ps = acc
                else:
                    ps = pmm.tile([128, CHUNK], F32, name="mm", tag="mm")
                    self.conv(chunk, diag, CRPE_TAPS[ct], 0, src, ps)
                nc.vector.scalar_tensor_tensor(
                    out=tmp, in0=ps, scalar=W["bcrpe"][:, ct:ct + 1],
                    in1=qT[ct][:, sl], op0=OP.add, op1=OP.mult)
                ps2 = pmm.tile([128, CHUNK], F32, name="mm", tag="mm")
                for h in range(2):
                    nc.tensor.matmul(ps2[h * 64:h * 64 + 64, :],
                                     kv[ct][h * 64:h * 64 + 64, :],
                                     qT[ct][h * 64:h * 64 + 64, sl],
                                     start=True, stop=True,
                                     tile_position=(h * 64, h * 64))
                nc.vector.tensor_add(out=attnT[ct][:, sl], in0=ps2, in1=tmp)
        self.dump(f"attnT_{img}", attnT)
        return attnT

    def proj(self, img, attnT, res):
        nc, W = self.nc, self.W
        pmm = self.pool("pmm", 3, space="PSUM")
        x0pb = [self.big(f"g2_{t}") for t in range(CT)]
        for co in range(CT):
            for chunk in range(NCHUNK):
                ps = pmm.tile([128, CHUNK], F32, name="mm", tag="mm")
                for ci in range(CT):
                    nc.tensor.matmul(ps,
                                     W[f"wproj{ci}"][:, co * 128:(co + 1) * 128],
                                     attnT[ci][:, bass.ts(chunk, CHUNK)],
                                     start=(ci == 0), stop=(ci == CT - 1))
                sl = bass.ts(chunk, CHUNK)
                nc.vector.scalar_tensor_tensor(
                    out=res[co][:, sl], in0=ps, scalar=W["bproj"][:, co:co + 1],
                    in1=res[co][:, sl], op0=OP.add, op1=OP.add)
                nc.vector.tensor_copy(out=x0pb[co][:, sl], in_=res[co][:, sl])
        self.dump(f"x0pT_{img}", res)
        return x0pb

    def ffn(self, img, y2, res):
        nc, W = self.nc, self.W
        pmm = self.pool("pmm", 3, space="PSUM")
        wfc1 = [self.big(f"g1_{ci}", BF16, [128, HID]) for ci in range(CT)]
        wfc2 = [self.big(f"g3_{kt}", BF16, [128, 4, C]) for kt in range(CT)]
        for ci in range(CT):
            self.dma(wfc1[ci], self.aps["wfc1"][ci * 128:(ci + 1) * 128, :])
        for kt in range(16):
            self.dma(wfc2[kt // 4][:, kt % 4, :],
                     self.aps["wfc2"][kt * 128:(kt + 1) * 128, :])
        for chunk in range(NCHUNK):
            sl = bass.ts(chunk, CHUNK)
            tags = ("g4_0", "g4_1") if chunk % 2 == 0 else ("g5_0", "g5_1")
            hdn_ab = [self.big(tags[0], BF16, [128, 8, CHUNK]),
                      self.big(tags[1], BF16, [128, 8, CHUNK])]
            hdn = lambda kt: hdn_ab[kt // 8][:, kt % 8, :]
            for ho in range(16):
                ps = pmm.tile([128, CHUNK], F32, name="mm", tag="mm")
                for ci in range(CT):
                    nc.tensor.matmul(ps, wfc1[ci][:, ho * 128:(ho + 1) * 128],
                                     y2[ci][:, sl],
                                     start=(ci == 0), stop=(ci == CT - 1))
                nc.scalar.activation(out=hdn(ho), in_=ps, func=AF.Gelu,
                                     bias=W["bfc1"][:, ho:ho + 1], scale=1.0)
            for co in range(CT):
                ps = pmm.tile([128, CHUNK], F32, name="mm", tag="mm")
                for kt in range(16):
                    nc.tensor.matmul(ps,
                                     wfc2[kt // 4][:, kt % 4,
                                                   co * 128:(co + 1) * 128],
                                     hdn(kt),
                                     start=(kt == 0), stop=(kt == 15))
                nc.vector.scalar_tensor_tensor(
                    out=res[co][:, sl], in0=ps, scalar=W["bfc2"][:, co:co + 1],
                    in1=res[co][:, sl], op0=OP.add, op1=OP.add)
        self.dump(f"outT_{img}", res)

    def transpose_out(self, img, res):
        nc, W = self.nc, self.W
        ptok = self.pool("ptok", 1)
        pst = self.pool("pst", 2, space="PSUM")
        for j in range(JT):
            rows = JW(j)
            ps = pst.tile([128, 512], F32, name="tpf", tag="tpf", bufs=3)
            for ct in range(CT):
                nc.tensor.transpose(ps[:rows, ct * 128:(ct + 1) * 128],
                                    res[ct][:, j * 128:j * 128 + rows],
                                    W["id_f32"])
            t = ptok.tile([128, C], F32, name=f"xtok{j % 4}", tag=f"xtok{j % 4}")
            nc.scalar.copy(out=t[:rows], in_=ps[:rows])
            self.dma(self.aps["out"][img, j * 128:j * 128 + rows, :], t[:rows])

    def image(self, img):
        res, xTb = self.transpose_in(img)
        x0b = self.cpe(img, res, xTb)
        x0s = self.ln(img, x0b, [f"g3_{t}" for t in range(CT)],
                      ["g5_0", "g5_1"])
        self.dump(f"x0s_{img}", x0s)
        qT, ekT, vT, recip = self.qkv(img, x0s)
        kv = self.kv(img, ekT, vT, recip)
        attnT = self.attn(img, qT, vT, kv)
        x0pb = self.proj(img, attnT, res)
        y2 = self.ln(img, x0pb, [f"g3_{t}" for t in range(CT)],
                     ["g5_0", "g5_1"])
        self.dump(f"y2_{img}", y2)
        self.ffn(img, y2, res)
        self.transpose_out(img, res)

    def build(self):
        self.load_weights()
        for img in range(BPC):
            self.image(img)
        for p in reversed(list(self.pools.values())):
            p.release()


DEBUG_TENSORS = []
for img in range(BPC):
    DEBUG_TENSORS += [
        (f"x0T_{img}", F32), (f"x0s_{img}", BF16), (f"qT_{img}", BF16),
        (f"ekT_{img}", BF16), (f"vT_{img}", BF16), (f"attnT_{img}", BF16),
        (f"x0pT_{img}", F32), (f"y2_{img}", BF16), (f"outT_{img}", F32),
    ]


def build_nc(debug=False):
    nc = bacc.Bacc("TRN2", target_bir_lowering=False, debug=False,
                   num_devices=NCORES)
    aps = {}
    aps["x"] = nc.dram_tensor("x", [BPC, NTOK, C], F32, kind="ExternalInput").ap()
    for name, shape, dt in WEIGHT_SPECS:
        aps[name] = nc.dram_tensor(name, shape, dt, kind="ExternalInput").ap()
    aps["out"] = nc.dram_tensor("out", [BPC, NTOK, C], F32,
                                kind="ExternalOutput").ap()
    if debug:
        for name, dt in DEBUG_TENSORS:
            aps[name] = nc.dram_tensor(name, [CT, 128, NTOK], dt,
                                       kind="ExternalOutput").ap()
        aps["kv_0"] = nc.dram_tensor("kv_0", [CT, 128, CHD], BF16,
                                     kind="ExternalOutput").ap()
    with tile.TileContext(nc) as tc:
        Builder(nc, tc, aps, debug).build()
    nc.compile()
    return nc


_CACHE = {}


def run(inputs, debug=False):
    x, w = _prep(inputs)
    key = "dbg" if debug else "plain"
    if key not in _CACHE:
        _CACHE[key] = build_nc(debug)
    nc = _CACHE[key]
    in_maps = []
    for c in range(NCORES):
        m = {"x": np.ascontiguousarray(x[c * BPC:(c + 1) * BPC])}
        m.update(w)
        in_maps.append(m)
    return bass_utils.run_bass_kernel_spmd(nc, in_maps,
                                           core_ids=list(range(NCORES)))


def kernel(**inputs):
    res = run(inputs)
    out = np.concatenate([res.results[c]["out"] for c in range(NCORES)], axis=0)
    return out.astype(np.float32)


# revision 29
# speedup vs baseline: 6360.4284x; 1.0565x over previous
"""Trainium2 Bass kernel for a CoaT-style decoder block (ConvPosEnc +
FactorAttn w/ ConvRelPosEnc + FFN), data-parallel over batch on 8 cores.

Layout: activations channel-major [C(part), N(free)]. Matmuls are
weight-stationary (lhsT = W[cin, cout]) so outputs stay channel-major; the
kv einsum uses PE-transposed token-major tiles. Depthwise convs run on the
PE as per-tap diagonal matmuls over spatially shifted access patterns.
Matmul inputs are bf16 (fp32 PSUM accumulation); the residual stream lives
in-place in fp32 tiles; softmax and LN statistics are fp32.

SBUF tag plan (static allocation = sum over tags): the big [128,3136]
tiles share tag groups whose tenants have disjoint lifetimes:
  res{ct}  f32 : x^T -> x0 -> x0+attn -> out  (in-place residual)
  g1{ct} bf16  : xTb -> qT -> wfc1
  g2{ct} bf16  : x0b/x0s(in-place LN) -> attnT -> x0pb/y2(in-place LN)
  g3{ct} bf16  : sq -> ekT -> sq2 -> wfc2
  g4{ct} bf16  : vT  (g4_0 also hosts hdn [128,16,448])
  g5{0,1} bf16 : rbc/mbc -> wqkv01/23 -> ektok/vtok -> rbc2/mbc2
"""

import numpy as np
import ml_dtypes

import concourse.bass as bass
import concourse.bacc as bacc
import concourse.tile as tile
import concourse.mybir as mybir
from concourse import bass_utils

F32 = mybir.dt.float32
BF16 = mybir.dt.bfloat16
AF = mybir.ActivationFunctionType
OP = mybir.AluOpType
AX = mybir.AxisListType

B, NTOK, C = 16, 3136, 512
HH = WW = 56
NHEADS, CHD = 8, 64
HID = 2048
NCORES = 8
BPC = B // NCORES          # images per core
CT = 4                     # 128-channel tiles in C
CHUNK = 448                # tokens per matmul psum chunk (8 image rows)
NCHUNK = NTOK // CHUNK     # 7
RPC = 8                    # image rows per chunk
JT = 25                    # 128-token blocks (last has 64)
EPS = 1e-6

bf16 = ml_dtypes.bfloat16


def _taps(k):
    """Center-first tap list (first matmul must cover the full psum chunk)."""
    p = k // 2
    out = [(0, 0)]
    for dy in range(-p, p + 1):
        for dx in range(-p, p + 1):
            if (dy, dx) != (0, 0):
                out.append((dy, dx))
    return out

TAPS3, TAPS5, TAPS7 = _taps(3), _taps(5), _taps(7)
CRPE_TAPS = [TAPS3, TAPS5, TAPS7, TAPS7]
CRPE_OFF = [0, 9, 34, 83]
CRPE_NTAP = 132


def _diag_pack(ntap, weight_cols):
    out = np.zeros((128, ntap, 128), np.float32)
    idx = np.arange(128)
    for t in range(ntap):
        out[idx, t, idx] = weight_cols[t]
    return out.astype(bf16)


def _prep(inputs):
    g = lambda k: np.asarray(inputs[k], np.float32)
    x = g("x")
    qkv_w, proj_w, proj_b = g("qkv_w"), g("proj_w"), g("proj_b")
    fc1_w, fc1_b, fc2_w, fc2_b = g("fc1_w"), g("fc1_b"), g("fc2_w"), g("fc2_b")
    ln1_w, ln1_b, ln2_w, ln2_b = g("ln1_w"), g("ln1_b"), g("ln2_w"), g("ln2_b")
    cpe_w, cpe_b = g("cpe_w"), g("cpe_b")
    w3, b3, w5, b5, w7, b7 = g("w3"), g("b3"), g("w5"), g("b5"), g("w7"), g("b7")

    wqkv = (ln1_w[:, None] * qkv_w).astype(bf16)
    bqkv = ln1_b @ qkv_w
    wfc1 = (ln2_w[:, None] * fc1_w).astype(bf16)
    bfc1 = fc1_b + ln2_b @ fc1_w

    tiles = lambda b: np.ascontiguousarray(b.reshape(-1, 128).T)

    dcpe = np.concatenate(
        [_diag_pack(9, np.stack([cpe_w[ct * 128:(ct + 1) * 128, 0, dy + 1, dx + 1]
                                 for (dy, dx) in TAPS3]))
         for ct in range(CT)], axis=1)

    def crpe_cols(ct, taps):
        cols = []
        for (dy, dx) in taps:
            w = np.zeros(128, np.float32)
            for p in range(128):
                vch = ct * 128 + p
                if vch < 128:
                    if abs(dy) <= 1 and abs(dx) <= 1:
                        w[p] = w3[vch, 0, dy + 1, dx + 1]
                elif vch < 320:
                    if abs(dy) <= 2 and abs(dx) <= 2:
                        w[p] = w5[vch - 128, 0, dy + 2, dx + 2]
                else:
                    w[p] = w7[vch - 320, 0, dy + 3, dx + 3]
            cols.append(w)
        return np.stack(cols)

    dcrpe = np.concatenate(
        [_diag_pack(len(CRPE_TAPS[ct]), crpe_cols(ct, CRPE_TAPS[ct]))
         for ct in range(CT)], axis=1)

    w = {
        "wqkv": wqkv, "wproj": proj_w.astype(bf16),
        "wfc1": wfc1, "wfc2": fc2_w.astype(bf16),
        "bqkv": tiles(bqkv), "bproj": tiles(proj_b),
        "bfc1": tiles(bfc1), "bfc2": tiles(fc2_b),
        "bcpe": tiles(cpe_b), "bcrpe": tiles(np.concatenate([b3, b5, b7])),
        "dcpe": dcpe, "dcrpe": dcrpe,
        "crpw": np.concatenate([crpe_cols(ct, CRPE_TAPS[ct]).T
                                for ct in range(CT)], axis=1).astype(np.float32),
        "id_f32": np.eye(128, dtype=np.float32),
        "id_bf": np.eye(128, dtype=np.float32).astype(bf16),
        "ones_col": np.ones((128, 1), bf16),
        "ones_row": np.ones((1, 128), bf16),
    }
    return x, w


WEIGHT_SPECS = [
    ("wqkv", [C, 3 * C], BF16), ("wproj", [C, C], BF16),
    ("wfc1", [C, HID], BF16), ("wfc2", [HID, C], BF16),
    ("bqkv", [128, 12], F32), ("bproj", [128, 4], F32),
    ("bfc1", [128, 16], F32), ("bfc2", [128, 4], F32),
    ("bcpe", [128, 4], F32), ("bcrpe", [128, 4], F32),
    ("dcpe", [128, 36, 128], BF16), ("dcrpe", [128, CRPE_NTAP, 128], BF16),
    ("crpw", [128, CRPE_NTAP], F32),
    ("id_f32", [128, 128], F32), ("id_bf", [128, 128], BF16),
    ("ones_col", [128, 1], BF16), ("ones_row", [1, 128], BF16),
]

# token groups of four 128-blocks (used by transposes); last group is [24]
JGROUPS = [list(range(4 * g, 4 * g + 4)) for g in range(6)] + [[24]]
JW = lambda j: 128 if j < 24 else 64


class Builder:
    def __init__(self, nc, tc, aps, debug):
        self.nc, self.tc, self.aps, self.debug = nc, tc, aps, debug
        self.pools = {}

    def pool(self, name, bufs, space="SBUF"):
        if name not in self.pools:
            self.pools[name] = self.tc.alloc_tile_pool(name=name, bufs=bufs,
                                                       space=space)
        return self.pools[name]

    def dma(self, out, in_):
        self.nc.sync.dma_start(out=out, in_=in_)

    def big(self, tag, dtype=BF16, shape=None):
        return self.pool("pbig", 1).tile(shape or [128, NTOK], dtype,
                                         name=tag, tag=tag)

    def load_weights(self):
        nc, aps = self.nc, self.aps
        pw = self.pool("pw", 1)
        W = {}
        for ci in range(CT):
            t = pw.tile([128, C], BF16, name=f"wproj{ci}", tag=f"wproj{ci}")
            self.dma(t, aps["wproj"][ci * 128:(ci + 1) * 128, :])
            W[f"wproj{ci}"] = t
        names = ["bqkv", "bproj", "bfc1", "bfc2", "bcpe", "bcrpe",
                 "id_f32", "id_bf", "ones_col", "ones_row"]
        if self.DVE_CONV:
            names.append("crpw")
        for nm in names:
            t = pw.tile(list(aps[nm].shape), aps[nm].dtype, name=nm, tag=nm)
            self.dma(t, aps[nm])
            W[nm] = t
        eps = pw.tile([128, 1], F32, name="eps", tag="eps")
        nc.vector.memset(eps, EPS)
        W["eps"] = eps
        self.W = W

    def dump(self, name, tiles):
        if not self.debug or name not in self.aps:
            return
        for i, t in enumerate(tiles):
            self.dma(self.aps[name][i], t)

    # ---------- stages ----------
    def transpose_in(self, img):
        """x[img] -> res (f32 channel-major) and xTb (bf16 copy)."""
        nc, W = self.nc, self.W
        ptok = self.pool("ptok", 1)
        pst = self.pool("pst", 2, space="PSUM")
        res = [self.big(f"res{ct}", F32) for ct in range(CT)]
        xTb = [self.big(f"g1_{ct}") for ct in range(CT)]
        for g, js in enumerate(JGROUPS):
            xtok = []
            for j in js:
                t = ptok.tile([128, C], F32, name=f"xtok{j % 5}", tag=f"xtok{j % 5}")
                self.dma(t[:JW(j)], self.aps["x"][img, j * 128:j * 128 + JW(j), :])
                xtok.append((t, JW(j)))
            for ct in range(CT):
                ps = pst.tile([128, 512], F32, name="tpf", tag="tpf", bufs=3)
                for i, (t, rows) in enumerate(xtok):
                    nc.tensor.transpose(ps[:, i * 128:i * 128 + rows],
                                        t[:rows, ct * 128:(ct + 1) * 128],
                                        W["id_f32"][:rows, :rows])
                width = sum(r for _, r in xtok)
                nc.scalar.copy(out=res[ct][:, g * 512:g * 512 + width],
                               in_=ps[:, :width])
        for ct in range(CT):
            nc.vector.tensor_copy(out=xTb[ct], in_=res[ct])
        return res, xTb

    def conv(self, chunk, segs, ntap, src_view, ps):
        """segs: list of (diag_tile, tap_off_in_tile, tap_list, global_t0)."""
        nc = self.nc
        psv = ps.rearrange("p (h w) -> p h w", h=RPC)
        r0 = chunk * RPC
        for diag, toff, taps, g0 in segs:
            for i, (dy, dx) in enumerate(taps):
                t = g0 + i
                y0 = max(r0, -dy)
                y1 = min(r0 + RPC, HH - max(0, dy))
                x0 = max(0, -dx)
                x1 = WW - max(0, dx)
                nc.tensor.matmul(
                    psv[:, y0 - r0:y1 - r0, x0:x1],
                    diag[:, toff + i, :],
                    src_view[:, y0 + dy:y1 + dy, x0 + dx:x1 + dx],
                    start=(t == 0), stop=(t == ntap - 1),
                    skip_group_check=True)

    def cpe(self, img, res, xTb):
        """res = res + dwconv3(xTb) + bias (in-place); x0b = bf16(res)."""
        nc, W = self.nc, self.W
        pdiag = self.pool("pdiag", 1)
        pmm = self.pool("pmm", 3, space="PSUM")
        x0b = [self.big(f"g2_{ct}") for ct in range(CT)]
        for ct in range(CT):
            tg = f"diag{ct % 2}"
            dcpe = pdiag.tile([128, 25, 128], BF16, name=tg, tag=tg)
            self.dma(dcpe[:, :9, :], self.aps["dcpe"][:, ct * 9:ct * 9 + 9, :])
            segs = [(dcpe, 0, TAPS3, 0)]
            src = xTb[ct].rearrange("p (h w) -> p h w", h=HH)
            for chunk in range(NCHUNK):
                ps = pmm.tile([128, CHUNK], F32, name="mm", tag="mm")
                self.conv(chunk, segs, 9, src, ps)
                sl = bass.ts(chunk, CHUNK)
                nc.vector.scalar_tensor_tensor(
                    out=res[ct][:, sl], in0=ps, scalar=W["bcpe"][:, ct:ct + 1],
                    in1=res[ct][:, sl], op0=OP.add, op1=OP.add)
                nc.vector.tensor_copy(out=x0b[ct][:, sl], in_=res[ct][:, sl])
        self.dump(f"x0T_{img}", res)
        return x0b

    def ln(self, img, xb, sq_tags, bc_tags):
        """Channel-major LN over xb (list of 4 bf16 tiles), applied IN-PLACE.
        xb becomes the normalized tensor (gamma/beta folded downstream)."""
        nc, W = self.nc, self.W
        psm = self.pool("psmall", 1)
        pstat = self.pool("pst", 2, space="PSUM")
        sq = [self.big(t) for t in sq_tags]
        for ct in range(CT):
            nc.scalar.square(out=sq[ct], in_=xb[ct])
        # per-token sums over channels -> [128, 25] token-tiled stats
        st = psm.tile([128, JT], F32, name="st", tag="st")
        s2t = psm.tile([128, JT], F32, name="s2t", tag="s2t")
        for dst, srcs in ((st, xb), (s2t, sq)):
            pst_cols = pstat.tile([128, 32], F32, name="stt", tag="tpf", bufs=3)
            for g, js in enumerate(JGROUPS):
                w = sum(JW(j) for j in js)
                ps = pstat.tile([1, 512], F32, name="srow", tag="tpf", bufs=3)
                for ct in range(CT):
                    nc.tensor.matmul(ps[:, :w], W["ones_col"],
                                     srcs[ct][:, g * 512:g * 512 + w],
                                     start=(ct == 0), stop=(ct == CT - 1))
                rowb = psm.tile([1, 512], F32, name="rowb", tag="rowb")
                nc.scalar.copy(out=rowb[:, :w], in_=ps[:, :w])
                for i, j in enumerate(js):
                    nc.tensor.transpose(pst_cols[:JW(j), j:j + 1],
                                        rowb[0:1, i * 128:i * 128 + JW(j)],
                                        W["id_f32"][0:1, 0:1])
            nc.vector.tensor_copy(out=dst, in_=pst_cols[:, :JT])
        ms = psm.tile([128, JT], F32, name="ms", tag="ms")
        var = psm.tile([128, JT], F32, name="var", tag="var")
        nc.vector.tensor_scalar_mul(out=ms, in0=st, scalar1=1.0 / C)
        nc.vector.tensor_scalar_mul(out=var, in0=s2t, scalar1=1.0 / C)
        nc.vector.tensor_mul(out=st, in0=ms, in1=ms)
        nc.vector.tensor_sub(out=var, in0=var, in1=st)
        nc.scalar.activation(out=var, in_=var, func=AF.Sqrt, bias=W["eps"],
                             scale=1.0)
        nc.vector.reciprocal(out=var, in_=var)
        rstd = var
        nc.vector.tensor_mul(out=ms, in0=ms, in1=var)
        mrs = ms
        # broadcast rstd/mrs along partitions: [128,25] -> row chunks -> K=1 mm
        rbc = self.big(bc_tags[0])
        mbc = self.big(bc_tags[1])
        for dst, src in ((rbc, rstd), (mbc, mrs)):
            for g, js in enumerate(JGROUPS):
                w = sum(JW(j) for j in js)
                psr = pstat.tile([1, 512], F32, name="srow", tag="tpf", bufs=3)
                off = 0
                for j in js:
                    nc.tensor.transpose(psr[0:1, off:off + JW(j)],
                                        src[:JW(j), j:j + 1],
                                        W["id_f32"][:JW(j), :JW(j)])
                    off += JW(j)
                rowb = psm.tile([1, 512], BF16, name="rowbb", tag="rowb")
                nc.scalar.copy(out=rowb[:, :w], in_=psr[:, :w])
                psb = pstat.tile([128, 512], F32, name="bc", tag="tpf", bufs=3)
                nc.tensor.matmul(psb[:, :w], W["ones_row"], rowb[0:1, :w],
                                 start=True, stop=True)
                nc.scalar.copy(out=dst[:, g * 512:g * 512 + w], in_=psb[:, :w])
        # apply in place, chunk-wise so downstream matmuls start early
        for ct in range(CT):
            for chunk in range(NCHUNK):
                sl = bass.ts(chunk, CHUNK)
                nc.vector.tensor_mul(out=xb[ct][:, sl], in0=xb[ct][:, sl],
                                     in1=rbc[:, sl])
                nc.vector.tensor_sub(out=xb[ct][:, sl], in0=xb[ct][:, sl],
                                     in1=mbc[:, sl])
        return xb

    def qkv(self, img, x0s):
        nc, W = self.nc, self.W
        psm = self.pool("psmall", 1)
        pmm = self.pool("pmm", 3, space="PSUM")
        wq = [self.big("g5_0", BF16, [128, 2, 3 * C]),
              self.big("g5_1", BF16, [128, 2, 3 * C])]
        for co in range(12):
            for ci in range(CT):
                self.dma(wq[ci // 2][:, ci % 2, co * 128:(co + 1) * 128],
                         self.aps["wqkv"][ci * 128:(ci + 1) * 128,
                                          co * 128:(co + 1) * 128])
        qT = [self.big(f"g1_{t}") for t in range(CT)]
        ekT = [self.big(f"g3_{t}") for t in range(CT)]
        vT = [self.big(f"g4_{t}") if t > 0 else
              self.big("g4_0", BF16, [128, 16, CHUNK]) for t in range(CT)]
        vT[0] = vT[0].rearrange("p a b -> p (a b)")[:, :NTOK]
        sep = [psm.tile([128, NCHUNK], F32, name=f"sep{t}", tag=f"sep{t}")
               for t in range(CT)]
        recip = [psm.tile([128, 1], F32, name=f"rec{t}", tag=f"rec{t}")
                 for t in range(CT)]
        for co in range(12):
            for chunk in range(NCHUNK):
                ps = pmm.tile([128, CHUNK], F32, name="mm", tag="mm")
                for ci in range(CT):
                    nc.tensor.matmul(ps, wq[ci // 2][:, ci % 2,
                                                     co * 128:(co + 1) * 128],
                                     x0s[ci][:, bass.ts(chunk, CHUNK)],
                                     start=(ci == 0), stop=(ci == CT - 1))
                bias = W["bqkv"][:, co:co + 1]
                sl = bass.ts(chunk, CHUNK)
                if co < 4:
                    nc.scalar.activation(out=qT[co][:, sl], in_=ps,
                                         func=AF.Identity, bias=bias, scale=1.0)
                elif co < 8:
                    t = co - 4
                    nc.scalar.activation(out=ekT[t][:, sl], in_=ps, func=AF.Exp,
                                         bias=bias, scale=1.0,
                                         accum_out=sep[t][:, chunk:chunk + 1])
                else:
                    nc.scalar.activation(out=vT[co - 8][:, sl], in_=ps,
                                         func=AF.Identity, bias=bias, scale=1.0)
        for t in range(CT):
            s = psm.tile([128, 1], F32, name=f"sume{t}", tag=f"sume{t}")
            nc.vector.tensor_reduce(out=s, in_=sep[t], axis=AX.X, op=OP.add)
            nc.vector.reciprocal(out=recip[t], in_=s)
        self.dump(f"qT_{img}", qT)
        self.dump(f"ekT_{img}", ekT)
        self.dump(f"vT_{img}", vT)
        return qT, ekT, vT, recip

    def kv(self, img, ekT, vT, recip):
        nc, W = self.nc, self.W
        psm = self.pool("psmall", 1)
        pst = self.pool("pst", 2, space="PSUM")
        kv = []
        scale = CHD ** -0.5
        for t in range(CT):
            ektok = self.big("g5_0", BF16, [128, JT, 128])
            vtok = self.big("g5_1", BF16, [128, JT, 128])
            for src, dst in ((ekT[t], ektok), (vT[t], vtok)):
                for g, js in enumerate(JGROUPS):
                    ps = pst.tile([128, 512], BF16, name="tpb", tag="tpb")
                    for i, j in enumerate(js):
                        nc.tensor.transpose(ps[:JW(j), i * 128:(i + 1) * 128],
                                            src[:, j * 128:j * 128 + JW(j)],
                                            W["id_bf"])
                    if len(js) == 4:
                        nc.vector.tensor_copy(
                            out=dst[:, js[0]:js[0] + 4, :].rearrange(
                                "p a b -> p (a b)"),
                            in_=ps[:, :512])
                    else:
                        for i, j in enumerate(js):
                            nc.vector.tensor_copy(
                                out=dst[:JW(j), j, :],
                                in_=ps[:JW(j), i * 128:(i + 1) * 128])
            ps = pst.tile([128, CHD], F32, name="kvps", tag="tpf", bufs=3)
            for h in range(2):
                for j in range(JT):
                    nc.tensor.matmul(
                        ps[h * 64:h * 64 + 64, :],
                        ektok[:JW(j), j, h * 64:h * 64 + 64],
                        vtok[:JW(j), j, h * 64:h * 64 + 64],
                        start=(j == 0), stop=(j == JT - 1),
                        tile_position=(0, h * 64))
            kvt = psm.tile([128, CHD], BF16, name=f"kv{t}", tag=f"kv{t}")
            nc.vector.tensor_scalar(out=kvt, in0=ps, scalar1=recip[t],
                                    scalar2=scale, op0=OP.mult, op1=OP.mult)
            kv.append(kvt)
        self.dump(f"kv_{img}", kv)
        return kv

    DVE_CONV = ()

    def conv_dve(self, ct, chunk, wcol, src_view, acc):
        nc = self.nc
        accv = acc.rearrange("p (h w) -> p h w", h=RPC)
        r0 = chunk * RPC
        for t, (dy, dx) in enumerate(CRPE_TAPS[ct]):
            y0 = max(r0, -dy)
            y1 = min(r0 + RPC, HH - max(0, dy))
            x0 = max(0, -dx)
            x1 = WW - max(0, dx)
            s = wcol[:, CRPE_OFF[ct] + t:CRPE_OFF[ct] + t + 1]
            iv = src_view[:, y0 + dy:y1 + dy, x0 + dx:x1 + dx]
            ov = accv[:, y0 - r0:y1 - r0, x0:x1]
            if t == 0:
                nc.vector.tensor_scalar_mul(out=acc, in0=iv, scalar1=s)
            else:
                nc.vector.scalar_tensor_tensor(out=ov, in0=iv, scalar=s,
                                               in1=ov, op0=OP.mult, op1=OP.add)

    def attn(self, img, qT, vT, kv):
        nc, W = self.nc, self.W
        pdiag = self.pool("pdiag", 1)
        psm = self.pool("psmall", 1)
        pmm = self.pool("pmm", 3, space="PSUM")
        attnT = [self.big(f"g2_{t}") for t in range(CT)]
        for ct in range(CT):
            ntap = len(CRPE_TAPS[ct])
            use_dve = ct in self.DVE_CONV
            if not use_dve:
                off = CRPE_OFF[ct]
                n1 = min(25, ntap)
                tg = f"diag{ct % 2}"
                dA = pdiag.tile([128, 25, 128], BF16, name=tg, tag=tg)
                self.dma(dA[:, :n1, :], self.aps["dcrpe"][:, off:off + n1, :])
                segs = [(dA, 0, CRPE_TAPS[ct][:n1], 0)]
                if ntap > n1:
                    tg2 = "diagB"
                    dB = pdiag.tile([128, 24, 128], BF16, name=tg2, tag=tg2)
                    self.dma(dB[:, :ntap - n1, :],
                             self.aps["dcrpe"][:, off + n1:off + ntap, :])
                    segs.append((dB, 0, CRPE_TAPS[ct][n1:], n1))
            src = vT[ct].rearrange("p (h w) -> p h w", h=HH)
            for chunk in range(NCHUNK):
                sl = bass.ts(chunk, CHUNK)
                tmp = psm.tile([128, CHUNK], BF16, name="tmp", tag="tmp")
                if use_dve:
                    acc = psm.tile([128, CHUNK], F32, name="dacc", tag="dacc")
                    self.conv_dve(ct, chunk, W["crpw"], src, acc)
                    ps = acc
                else:
                    ps = pmm.tile([128, CHUNK], F32, name="mm", tag="mm")
                    self.conv(chunk, segs, ntap, src, ps)
                nc.vector.scalar_tensor_tensor(
                    out=tmp, in0=ps, scalar=W["bcrpe"][:, ct:ct + 1],
                    in1=qT[ct][:, sl], op0=OP.add, op1=OP.mult)
                ps2 = pmm.tile([128, CHUNK], F32, name="mm", tag="mm")
                for h in range(2):
                    nc.tensor.matmul(ps2[h * 64:h * 64 + 64, :],
                                     kv[ct][h * 64:h * 64 + 64, :],
                                     qT[ct][h * 64:h * 64 + 64, sl],
                                     start=True, stop=True,
                                     tile_position=(h * 64, h * 64))
                nc.vector.tensor_add(out=attnT[ct][:, sl], in0=ps2, in1=tmp)
        self.dump(f"attnT_{img}", attnT)
        return attnT

    def proj(self, img, attnT, res):
        nc, W = self.nc, self.W
        pmm = self.pool("pmm", 3, space="PSUM")
        x0pb = [self.big(f"g2_{t}") for t in range(CT)]
        for co in range(CT):
            for chunk in range(NCHUNK):
                ps = pmm.tile([128, CHUNK], F32, name="mm", tag="mm")
                for ci in range(CT):
                    nc.tensor.matmul(ps,
                                     W[f"wproj{ci}"][:, co * 128:(co + 1) * 128],
                                     attnT[ci][:, bass.ts(chunk, CHUNK)],
                                     start=(ci == 0), stop=(ci == CT - 1))
                sl = bass.ts(chunk, CHUNK)
                nc.vector.scalar_tensor_tensor(
                    out=res[co][:, sl], in0=ps, scalar=W["bproj"][:, co:co + 1],
                    in1=res[co][:, sl], op0=OP.add, op1=OP.add)
                nc.vector.tensor_copy(out=x0pb[co][:, sl], in_=res[co][:, sl])
        self.dump(f"x0pT_{img}", res)
        return x0pb

    def ffn(self, img, y2, res):
        nc, W = self.nc, self.W
        pmm = self.pool("pmm", 3, space="PSUM")
        wfc1 = [self.big(f"g1_{ci}", BF16, [128, HID]) for ci in range(CT)]
        wfc2 = [self.big(f"g3_{kt}", BF16, [128, 4, C]) for kt in range(CT)]
        for ci in range(CT):
            self.dma(wfc1[ci], self.aps["wfc1"][ci * 128:(ci + 1) * 128, :])
        for kt in range(16):
            self.dma(wfc2[kt // 4][:, kt % 4, :],
                     self.aps["wfc2"][kt * 128:(kt + 1) * 128, :])
        for chunk in range(NCHUNK):
            sl = bass.ts(chunk, CHUNK)
            tags = ("g4_0", "g4_1") if chunk % 2 == 0 else ("g5_0", "g5_1")
            hdn_ab = [self.big(tags[0], BF16, [128, 8, CHUNK]),
                      self.big(tags[1], BF16, [128, 8, CHUNK])]
            hdn = lambda kt: hdn_ab[kt // 8][:, kt % 8, :]
            for ho in range(16):
                ps = pmm.tile([128, CHUNK], F32, name="mm", tag="mm")
                for ci in range(CT):
                    nc.tensor.matmul(ps, wfc1[ci][:, ho * 128:(ho + 1) * 128],
                                     y2[ci][:, sl],
                                     start=(ci == 0), stop=(ci == CT - 1))
                nc.scalar.activation(out=hdn(ho), in_=ps, func=AF.Gelu,
                                     bias=W["bfc1"][:, ho:ho + 1], scale=1.0)
            for co in range(CT):
                ps = pmm.tile([128, CHUNK], F32, name="mm", tag="mm")
                for kt in range(16):
                    nc.tensor.matmul(ps,
                                     wfc2[kt // 4][:, kt % 4,
                                                   co * 128:(co + 1) * 128],
                                     hdn(kt),
                                     start=(kt == 0), stop=(kt == 15))
                nc.vector.scalar_tensor_tensor(
                    out=res[co][:, sl], in0=ps, scalar=W["bfc2"][:, co:co + 1],
                    in1=res[co][:, sl], op0=OP.add, op1=OP.add)
        self.dump(f"outT_{img}", res)

    def transpose_out(self, img, res):
        nc, W = self.nc, self.W
        ptok = self.pool("ptok", 1)
        pst = self.pool("pst", 2, space="PSUM")
        for j in range(JT):
            rows = JW(j)
            ps = pst.tile([128, 512], F32, name="tpf", tag="tpf", bufs=3)
            for ct in range(CT):
                nc.tensor.transpose(ps[:rows, ct * 128:(ct + 1) * 128],
                                    res[ct][:, j * 128:j * 128 + rows],
                                    W["id_f32"])
            t = ptok.tile([128, C], F32, name=f"xtok{j % 4}", tag=f"xtok{j % 4}")
            nc.scalar.copy(out=t[:rows], in_=ps[:rows])
            self.dma(self.aps["out"][img, j * 128:j * 128 + rows, :], t[:rows])

    def image(self, img):
        res, xTb = self.transpose_in(img)
        x0b = self.cpe(img, res, xTb)
        x0s = self.ln(img, x0b, [f"g3_{t}" for t in range(CT)],
                      ["g5_0", "g5_1"])
        self.dump(f"x0s_{img}", x0s)
        qT, ekT, vT, recip = self.qkv(img, x0s)
        kv = self.kv(img, ekT, vT, recip)
        attnT = self.attn(img, qT, vT, kv)
        x0pb = self.proj(img, attnT, res)
        y2 = self.ln(img, x0pb, [f"g3_{t}" for t in range(CT)],
                     ["g5_0", "g5_1"])
        self.dump(f"y2_{img}", y2)
        self.ffn(img, y2, res)
        self.transpose_out(img, res)

    def build(self):
        self.load_weights()
        for img in range(BPC):
            self.image(img)
        for p in reversed(list(self.pools.values())):
            p.release()


DEBUG_TENSORS = []
for img in range(BPC):
    DEBUG_TENSORS += [
        (f"x0T_{img}", F32), (f"x0s_{img}", BF16), (f"qT_{img}", BF16),
        (f"ekT_{img}", BF16), (f"vT_{img}", BF16), (f"attnT_{img}", BF16),
        (f"x0pT_{img}", F32), (f"y2_{img}", BF16), (f"outT_{img}", F32),
    ]


def build_nc(debug=False):
    nc = bacc.Bacc("TRN2", target_bir_lowering=False, debug=False,
                   num_devices=NCORES)
    aps = {}
    aps["x"] = nc.dram_tensor("x", [BPC, NTOK, C], F32, kind="ExternalInput").ap()
    for name, shape, dt in WEIGHT_SPECS:
        aps[name] = nc.dram_tensor(name, shape, dt, kind="ExternalInput").ap()
    aps["out"] = nc.dram_tensor("out", [BPC, NTOK, C], F32,
                                kind="ExternalOutput").ap()
    if debug:
        for name, dt in DEBUG_TENSORS:
            aps[name] = nc.dram_tensor(name, [CT, 128, NTOK], dt,
                                       kind="ExternalOutput").ap()
        aps["kv_0"] = nc.dram_tensor("kv_0", [CT, 128, CHD], BF16,
                                     kind="ExternalOutput").ap()
    with tile.TileContext(nc) as tc:
        Builder(nc, tc, aps, debug).build()
    nc.compile()
    return nc


_CACHE = {}


def run(inputs, debug=False):
    x, w = _prep(inputs)
    key = "dbg" if debug else "plain"
    if key not in _CACHE:
        _CACHE[key] = build_nc(debug)
    nc = _CACHE[key]
    in_maps = []
    for c in range(NCORES):
        m = {"x": np.ascontiguousarray(x[c * BPC:(c + 1) * BPC])}
        m.update(w)
        in_maps.append(m)
    return bass_utils.run_bass_kernel_spmd(nc, in_maps,
                                           core_ids=list(range(NCORES)))


def kernel(**inputs):
    res = run(inputs)
    out = np.concatenate([res.results[c]["out"] for c in range(NCORES)], axis=0)
    return out.astype(np.float32)
